# revision 7
# baseline (speedup 1.0000x reference)
"""GCN2 (nn_GCN2_42331197669873) Bass kernel for 8 TRN2 NeuronCores.

v2 design (vs v1: gather+scatter_add both via SWDGE on one queue):

Graph/data parallel: nodes sharded row-wise across 8 cores (12500 each).
Per layer:
  - AllGather node features (bf16, node-major) into x_rep; each core's
    AllGather payload carries 47 extra rows: its partial class-center sums,
    so no separate AllReduce is needed (partials summed locally after AG).
  - Sparse propagate, pull form, per core over its dst shard:
      * edges sorted by (src window, dst); 4 windows of 2 core-blocks each
        keep gather indices int16-addressable.
      * SWDGE dma_gather of x_rep rows in 1024-token calls, round-robined
        over 4 SWDGE queues (4 queues ~2.9ns/token vs 8.4 on one; >1024
        tokens per call wedges the Q7).
      * dst-side segment-sum via PE matmul instead of dma_scatter_add:
        for each 128-token column, agg[:, d0:d0+128] += gcol^T @ S where
        S[t, d-d0] = 0.45*edge_weight (host-precomputed bf16, streamed from
        DRAM; identical across layers). Columns sit at fixed dst offsets
        d0 = min(s*c, NS-128) with a stride s shared by all cores (SPMD);
        tokens are greedily packed into eligible columns on host.
      * agg is SBUF-resident [H, 12544] f32, initialized by DMA from
        xinit = 0.45*x + 0.1*x0 (written tile-wise by the previous layer's
        dense pass), so u = agg + p@r directly.
  - Dense pass per dst tile: u = agg_slice + r_cls@p; out = u @ Wc' with
    Wc' = (1-beta)I + beta*conv_w (folded on host); relu; writes
    next-layer tensors (feature-major bf16 + node-major bf16 + xinit f32)
    and accumulates the class-center partial for the next AllGather.
  - Last layer fuses lin1 instead of writing next-layer tensors.

kernel(**inputs) takes FULL unsharded inputs, returns FULL [100000, 47].
"""

import numpy as np
import ml_dtypes

from concourse import bass, bacc, tile, mybir, bass_utils
from concourse import library_config
from concourse.mybir import AxisListType
import concourse.tile_sem_assignment as _tsa
from concourse import bass_isa as _bisa

# Tile round-robins Pool-engine DMAs over all DMASW sem lanes ignoring
# queue_num; mixing SWDGE queues on one lane breaks its in-order-completion
# assumption. Segregate lanes by queue: queue q -> lanes [2q, 2q+2).
_NQ = 4
_orig_assign_tick = _tsa.TileClockTick._assign_tick


def _assign_tick_qsplit(self, inst):
    if (isinstance(inst, _tsa.DMAInst)
            and inst.engine == mybir.EngineType.Pool
            and not isinstance(inst, _bisa.UserSyncedRemoteDMADescs)
            and self.swdge_sem_count >= _NQ):
        qn = getattr(inst, "queue_num", 0) or 0
        per = self.swdge_sem_count // _NQ
        if not hasattr(self, "_qrr"):
            self._qrr = {}
        r = self._qrr.get(qn, 0)
        self._qrr[qn] = r + 1
        self.next_sw_dma_idx = (qn % _NQ) * per + r % per
    return _orig_assign_tick(self, inst)


_tsa.TileClockTick._assign_tick = _assign_tick_qsplit

F32 = mybir.dt.float32
BF16 = mybir.dt.bfloat16
I16 = mybir.dt.int16


class Cfg:
    def __init__(self, N=100000, E=800000, C=47, H=128, ncores=8,
                 L=4, alpha=0.1, theta=0.5, rsl=0.5):
        self.N, self.E, self.C, self.H = N, E, C, H
        self.ncores = ncores
        self.NS = N // ncores                 # nodes per core (12500)
        self.NT = (self.NS + 127) // 128      # dst tiles per core (98)
        self.NTP = self.NT * 128              # padded shard rows (12544)
        self.B = self.NS + 100                # AG block: data + 2*47 cpart + pad
        self.nwin = 4                         # src windows (2 core-blocks each)
        self.L, self.alpha, self.theta, self.rsl = L, alpha, theta, rsl


DEF = Cfg()

# set by kernel()/test before build_nc: (stride, ncols, ncalls)
SCHED = None


# ----------------------------------------------------------------------
# host-side edge preprocessing
# ----------------------------------------------------------------------

def _greedy_cols(d_sorted, d0, cap=128):
    """Assign dst-sorted tokens to columns; col c covers [d0[c], d0[c]+128).

    Returns col ids or None if infeasible."""
    C = len(d0)
    fill = np.zeros(C, np.int64)
    cols = np.empty(len(d_sorted), np.int64)
    nxt = 0  # all cols < nxt are full
    # c_lo for each token: first column covering d
    for i, d in enumerate(d_sorted):
        c = np.searchsorted(d0, d - 127, side="left")
        c = max(c, nxt)
        while c < C and (fill[c] >= cap or d0[c] + 127 < d):
            if fill[c] >= cap and c == nxt:
                nxt += 1
            c += 1
        if c >= C or d0[c] > d:
            return None
        cols[i] = c
        fill[c] += 1
    return cols


def _prep_edges(cfg, edge_index, edge_weight):
    """Token layout for the matmul-segment-sum propagate.

    Returns (gidx [nc, nwin, ncalls, 128, 64] i16,
             smat [nc, nwin, ncols, 128, 128] bf16,
             sched (stride, ncols, ncalls))."""
    c = cfg
    src = np.asarray(edge_index[0], np.int64)
    dst = np.asarray(edge_index[1], np.int64)
    ew = (np.asarray(edge_weight, np.float32)
          * (1.0 - c.alpha) * (1.0 - c.rsl))          # fold 0.45
    nc, NS, B, nwin = c.ncores, c.NS, c.B, c.nwin
    WN = NS * 2                                       # nodes per window

    per_core = []
    for ci in range(nc):
        m = (dst >= ci * NS) & (dst < (ci + 1) * NS)
        s_c, d_c, w_c = src[m], dst[m] - ci * NS, ew[m]
        wi = s_c // WN
        order = np.lexsort((d_c, wi))
        per_core.append((s_c[order], d_c[order], w_c[order], wi[order]))

    # shared column schedule: try decreasing strides until all fit
    for stride in (64, 60, 56, 48, 40, 32):
        C = int(np.ceil((NS - 128) / stride)) + 1
        C = ((C + 7) // 8) * 8                        # whole 1024-tok calls
        d0 = np.minimum(stride * np.arange(C), NS - 128)
        all_cols = []
        ok = True
        for ci in range(nc):
            s_c, d_c, w_c, wi = per_core[ci]
            cols_ci = []
            for w in range(nwin):
                mw = wi == w
                cw = _greedy_cols(d_c[mw], d0)
                if cw is None:
                    ok = False
                    break
                cols_ci.append(cw)
            if not ok:
                break
            all_cols.append(cols_ci)
        if ok:
            break
    assert ok, "no feasible column stride"
    ncalls = C // 8

    gidx = np.zeros((nc, nwin, ncalls, 128, 64), np.int16)
    smat = np.zeros((nc, nwin, C, 128, 128), ml_dtypes.bfloat16)
    for ci in range(nc):
        s_c, d_c, w_c, wi = per_core[ci]
        for w in range(nwin):
            mw = wi == w
            s_w, d_w, w_w = s_c[mw], d_c[mw], w_c[mw]
            cols = all_cols[ci][w]
            # slot within column = arrival order (tokens are dst-sorted)
            order = np.argsort(cols, kind="stable")
            s_w, d_w, w_w, cols = s_w[order], d_w[order], w_w[order], cols[order]
            slot = np.arange(len(cols)) - np.searchsorted(cols, cols)
            # gather idx relative to window base (2w*B): core block + offset
            cc = s_w // NS
            rel = (cc - 2 * w) * B + (s_w - cc * NS)
            assert rel.min() >= 0 and rel.max() < 2 * B
            # S matrix entries
            smat[ci, w].reshape(-1)[
                (cols * 128 + slot) * 128 + (d_w - d0[cols])] = w_w
            # gather index layout: token t = (col%8)*128 + slot in call col//8
            call = cols // 8
            t = (cols % 8) * 128 + slot
            row0, col16 = t % 16, t // 16
            for g in range(8):
                gidx[ci, w, call, row0 + 16 * g, col16] = rel.astype(np.int16)
    return gidx, smat, (stride, C, ncalls, d0)


# ----------------------------------------------------------------------
# device program
# ----------------------------------------------------------------------

DBG_LI = 0


def build_nc(cfg):
    c = cfg
    nc = bacc.Bacc(None, target_bir_lowering=False, debug=False,
                   num_swdge_queues=_NQ)
    NT, NS, NTP, B, C_, H, L = c.NT, c.NS, c.NTP, c.B, c.C, c.H, c.L
    stride, NCOL, NCALL, D0 = SCHED
    nwin = c.nwin

    def dram_in(name, shape, dt=F32):
        return nc.declare_dram_parameter(name, shape, dt, isOutput=False)

    xin_t = dram_in("xin_t", [H, NS])
    d_t = dram_in("d_t", [NT, 128, C_])
    p_t = dram_in("p_t", [NT, C_, 128])
    gidx = dram_in("gidx", [nwin, NCALL, 128, 64], I16)
    smat = dram_in("smat", [nwin, NCOL, 128, 128], BF16)
    lin0w = dram_in("lin0w", [H, H])
    lin0b = dram_in("lin0b", [H, 1])
    lin1w = dram_in("lin1w", [H, C_])
    lin1b = dram_in("lin1b", [C_, 1])
    convw = dram_in("convw", [L, H, H])
    cma = dram_in("cma", [C_, C_])
    cmat = dram_in("cmat", [C_, C_])
    i47 = dram_in("i47", [C_, C_])
    ident = dram_in("ident", [128, 128])
    out_t = nc.declare_dram_parameter("out_t", [C_, NS], F32, isOutput=True)
    dbg_cen = nc.declare_dram_parameter("dbg_cen", [C_, H], F32, isOutput=True)
    dbg_rcls = nc.declare_dram_parameter("dbg_rcls", [C_, H], F32,
                                         isOutput=True)
    dbg_agg = nc.declare_dram_parameter("dbg_agg", [H, 1024], F32,
                                        isOutput=True)

    # internal DRAM
    x_rep = nc.dram_tensor("x_rep", [c.ncores * B, H], BF16,
                           addr_space="Shared")
    x_shp = [nc.dram_tensor(f"x_shp{i}", [B, H], BF16) for i in range(2)]
    x0s_sh = nc.dram_tensor("x0s_sh", [NTP, H], BF16)      # 0.1*x0 node-major
    xinit = [nc.dram_tensor(f"xinit{i}", [H, NTP], F32) for i in range(2)]
    x0sc_fm = nc.dram_tensor("x0sc_fm", [H, NTP], F32)     # 0.1*x0 f-major

    rg = [list(range(c.ncores))]

    def tsize(t):
        return min(128, NS - t * 128)

    with tile.TileContext(nc) as tc:
        nc.gpsimd.load_library(library_config.mlp)
        with (
            tc.tile_pool(name="const", bufs=1) as cpool,
            tc.tile_pool(name="sb", bufs=3) as pool,
            tc.tile_pool(name="gt", bufs=6) as gpool,
            tc.tile_pool(name="st", bufs=6) as spool,
            tc.tile_pool(name="ps", bufs=3, space="PSUM") as psum,
            tc.tile_pool(name="pst", bufs=1, space="PSUM") as pstr,
            tc.tile_pool(name="psc", bufs=3, space="PSUM") as pscol,
            tc.tile_pool(name="psacc", bufs=1, space="PSUM") as psacc,
        ):
            # ---- resident constants ----
            lin0w_sb = cpool.tile([H, H], F32)
            nc.sync.dma_start(lin0w_sb[:], lin0w[:, :])
            lin0b_sb = cpool.tile([H, 1], F32)
            nc.sync.dma_start(lin0b_sb[:], lin0b[:, :])
            lin1w_sb = cpool.tile([H, C_], F32)
            nc.sync.dma_start(lin1w_sb[:], lin1w[:, :])
            lin1b_sb = cpool.tile([C_, 1], F32)
            nc.sync.dma_start(lin1b_sb[:], lin1b[:, :])
            convw_sb = cpool.tile([H, L * H], F32)
            for i in range(L):
                nc.sync.dma_start(convw_sb[:, i * H:(i + 1) * H], convw[i])
            cma_sb = cpool.tile([C_, C_], F32)
            nc.sync.dma_start(cma_sb[:], cma[:, :])
            cmat_sb = cpool.tile([C_, C_], F32)
            nc.sync.dma_start(cmat_sb[:], cmat[:, :])
            i47_sb = cpool.tile([C_, C_], F32)
            nc.sync.dma_start(i47_sb[:], i47[:, :])
            ident_sb = cpool.tile([128, 128], F32)
            nc.sync.dma_start(ident_sb[:], ident[:, :])
            identb_sb = cpool.tile([128, 128], BF16)
            nc.vector.tensor_copy(identb_sb[:], ident_sb[:])
            lin1wb_sb = cpool.tile([H, C_], BF16)
            nc.vector.tensor_copy(lin1wb_sb[:], lin1w_sb[:])
            b55 = cpool.tile([H, 1], F32)
            nc.vector.tensor_scalar(b55[:], lin0b_sb[:], 0.55, None,
                                    mybir.AluOpType.mult)
            b10 = cpool.tile([H, 1], F32)
            nc.vector.tensor_scalar(b10[:], lin0b_sb[:], 0.1, None,
                                    mybir.AluOpType.mult)
            agg = cpool.tile([H, NTP], F32)

            # ---- lin0: x0 = relu(x @ W0 + b0) ----
            psA = psacc.tile([C_, H], F32, tag="cen")
            for t in range(NT):
                P = tsize(t)
                xi = pool.tile([H, 128], F32, tag="xi")
                nc.sync.dma_start(xi[:, :P], xin_t[:, t * 128:t * 128 + P])
                ps0 = psum.tile([H, 128], F32, tag="b")
                nc.tensor.matmul(ps0[:, :P], lin0w_sb[:], xi[:, :P],
                                 start=True, stop=True)
                # feature-major stores: 0.55*x0 (agg init L0), 0.1*x0,
                # 0.45*x0 (cur bf16), plus node-major bf16 copies
                xi0 = pool.tile([H, 128], F32, tag="xi0")
                nc.scalar.activation(xi0[:, :P], ps0[:, :P],
                                     mybir.ActivationFunctionType.Relu,
                                     bias=b55[:, 0:1], scale=0.55)
                nc.sync.dma_start(xinit[0][:, t * 128:t * 128 + P],
                                  xi0[:, :P])
                x1 = pool.tile([H, 128], F32, tag="x1")
                nc.scalar.activation(x1[:, :P], ps0[:, :P],
                                     mybir.ActivationFunctionType.Relu,
                                     bias=b10[:, 0:1], scale=0.1)
                nc.sync.dma_start(x0sc_fm[:, t * 128:t * 128 + P], x1[:, :P])
                # node-major: x0 bf16 (AG payload) and 0.1*x0 bf16
                x0b = pool.tile([H, 128], BF16, tag="x0b")
                nc.scalar.activation(x0b[:, :P], ps0[:, :P],
                                     mybir.ActivationFunctionType.Relu,
                                     bias=lin0b_sb[:, 0:1])
                pst = pstr.tile([128, 128], BF16, tag="bb")
                nc.tensor.transpose(pst[:P, :], x0b[:, :P], identb_sb[:])
                x0n = pool.tile([128, H], BF16, tag="x0n")
                nc.vector.tensor_copy(x0n[:P, :], pst[:P, :])
                nc.sync.dma_start(x_shp[0][t * 128:t * 128 + P, :], x0n[:P, :])
                x0ns = pool.tile([128, H], BF16, tag="x0ns")
                nc.vector.tensor_scalar(x0ns[:P, :], x0n[:P, :], 0.1, None,
                                        mybir.AluOpType.mult)
                nc.sync.dma_start(x0s_sh[t * 128:t * 128 + P, :],
                                  x0ns[:P, :])
                # center partial for layer 0: s = 1.1 * x0
                s0 = pool.tile([128, H], F32, tag="s0")
                nc.vector.tensor_scalar(s0[:P, :], x0n[:P, :], 1.1, None,
                                        mybir.AluOpType.mult)
                dt_ = pool.tile([128, C_], F32, tag="dt")
                nc.scalar.dma_start(dt_[:P, :], d_t[t][:P, :])
                nc.tensor.matmul(psA[:], dt_[:P, :], s0[:P, :],
                                 start=(t == 0), stop=(t == NT - 1))
            cp0 = pool.tile([C_, H], BF16, tag="cp")
            nc.vector.tensor_copy(cp0[:], psA[:])
            nc.sync.dma_start(x_shp[0][NS:NS + C_, :], cp0[:])
            cp0l = pool.tile([C_, H], BF16, tag="cpl")
            nc.vector.tensor_sub(cp0l[:], psA[:], cp0[:])
            nc.sync.dma_start(x_shp[0][NS + C_:NS + 2 * C_, :], cp0l[:])

            # ---- layers ----
            qrr = 0
            for li in range(L):
                cur_shp = x_shp[li % 2]
                nxt_shp = x_shp[(li + 1) % 2]

                nc.gpsimd.collective_compute(
                    "AllGather", mybir.AluOpType.bypass, replica_groups=rg,
                    ins=[cur_shp.ap().opt()], outs=[x_rep.ap().opt()],
                )

                # agg init = 0.45*x + 0.1*x0 (f32, feature-major)
                nc.scalar.dma_start(agg[:, :], xinit[li % 2][:, :])

                # centers from AG payload (hi+lo bf16 pairs -> ~f32)
                cen = pool.tile([C_, H], F32, tag="cen_sb")
                cp_a = pool.tile([C_, H], BF16, tag="cpa")
                nc.sync.dma_start(cp_a[:], x_rep[NS:NS + C_, :])
                cp_b = pool.tile([C_, H], BF16, tag="cpb")
                nc.sync.dma_start(cp_b[:], x_rep[NS + C_:NS + 2 * C_, :])
                nc.vector.tensor_add(cen[:], cp_a[:], cp_b[:])
                for ci in range(1, c.ncores):
                    for half in range(2):
                        cp_i = pool.tile([C_, H], BF16, tag="cpi")
                        r0 = ci * B + NS + half * C_
                        nc.sync.dma_start(cp_i[:], x_rep[r0:r0 + C_, :])
                        nc.vector.tensor_add(cen[:], cen[:], cp_i[:])

                # r_cls from centers (Gram trick); cma pre-scaled by 0.45
                pst = psum.tile([128, 128], F32, tag="b")
                nc.tensor.transpose(pst[:, :C_], cen[:], ident_sb[:C_, :C_])
                cT = pool.tile([128, C_], F32, tag="cT")
                nc.vector.tensor_copy(cT[:], pst[:, :C_])
                psg = psum.tile([128, 128], F32, tag="b")
                nc.tensor.matmul(psg[:C_, :C_], cT[:], cT[:], start=True,
                                 stop=True)
                g = pool.tile([C_, C_], F32, tag="gg")
                nc.vector.tensor_copy(g[:], psg[:C_, :C_])
                gd = pool.tile([C_, C_], F32, tag="gd")
                nc.vector.tensor_mul(gd[:], g[:], i47_sb[:])
                n2 = pool.tile([C_, 1], F32, tag="n2")
                nc.vector.reduce_sum(n2[:], gd[:], AxisListType.X)
                t1 = pool.tile([C_, C_], F32, tag="t1")
                nc.vector.tensor_scalar(t1[:], g[:], -1.0, n2[:, 0:1],
                                        mybir.AluOpType.mult,
                                        mybir.AluOpType.add)
                ps1 = psum.tile([128, 128], F32, tag="b")
                nc.tensor.transpose(ps1[:C_, :C_], t1[:], ident_sb[:C_, :C_])
                nrm = pool.tile([C_, C_], F32, tag="nrm")
                nc.vector.tensor_add(nrm[:], t1[:], ps1[:C_, :C_])
                nc.vector.tensor_relu(nrm[:], nrm[:])
                nc.vector.tensor_add(nrm[:], nrm[:], i47_sb[:])
                rn = pool.tile([C_, C_], F32, tag="rn")
                nc.scalar.sqrt(rn[:], nrm[:])
                inv = pool.tile([C_, C_], F32, tag="inv")
                nc.vector.reciprocal(inv[:], rn[:])
                amat = pool.tile([C_, C_], F32, tag="amat")
                nc.vector.tensor_mul(amat[:], cma_sb[:], inv[:])
                atm = pool.tile([C_, C_], F32, tag="atm")
                nc.vector.tensor_mul(atm[:], cmat_sb[:], inv[:])
                rs = pool.tile([C_, 1], F32, tag="rs")
                nc.vector.reduce_sum(rs[:], amat[:], AxisListType.X)
                psm = psum.tile([128, 128], F32, tag="b")
                nc.tensor.matmul(psm[:C_, :], atm[:], cen[:], start=True,
                                 stop=True)
                rcls = pool.tile([C_, H], F32, tag="rcls")
                nc.vector.tensor_scalar(rcls[:], cen[:], rs[:, 0:1], None,
                                        mybir.AluOpType.mult)
                nc.vector.tensor_sub(rcls[:], rcls[:], psm[:C_, :])
                if li == DBG_LI:
                    nc.sync.dma_start(dbg_cen[:, :], cen[:])
                    nc.sync.dma_start(dbg_rcls[:, :], rcls[:])

                # - propagate: gather + segment matmul -
                for w in range(nwin):
                    wbase = 2 * w * B
                    for k in range(NCALL):
                        gi = spool.tile([128, 64], I16, tag="gi")
                        nc.scalar.dma_start(gi[:], gidx[w, k])
                        st = spool.tile([128, 8, 128], BF16, tag="st")
                        nc.sync.dma_start(st[:, :, :],
                                          smat[w, k * 8:k * 8 + 8, :, :]
                                          .rearrange("a b c -> b a c"))
                        gt = gpool.tile([128, 8, 128], BF16, tag="g")
                        nc.gpsimd.dma_gather(
                            gt[:, :, :], x_rep[wbase:wbase + 2 * B, :],
                            gi[:, :], num_idxs=1024, num_idxs_reg=1024,
                            elem_size=H, queue_num=qrr % _NQ,
                        )
                        qrr += 1
                        for j in range(8):
                            col = k * 8 + j
                            d0 = int(D0[col])
                            psC = pscol.tile([H, 128], F32, tag="pc")
                            nc.tensor.matmul(psC[:, :], gt[:, j, :],
                                             st[:, j, :],
                                             start=True, stop=True)
                            nc.vector.tensor_add(agg[:, d0:d0 + 128],
                                                 agg[:, d0:d0 + 128],
                                                 psC[:, :])

                if li == DBG_LI:
                    nc.sync.dma_start(dbg_agg[:, :], agg[:, 0:1024])
                # - dense pass per dst tile -
                last = li == L - 1
                for t in range(NT):
                    P = tsize(t)
                    pt = pool.tile([C_, 128], F32, tag="pt")
                    nc.sync.dma_start(pt[:], p_t[t])
                    ps1b = psum.tile([H, 128], F32, tag="b")
                    nc.tensor.matmul(ps1b[:, :P], rcls[:], pt[:, :P],
                                     start=True, stop=True)
                    u = pool.tile([H, 128], F32, tag="u")
                    nc.vector.tensor_add(u[:, :P],
                                         agg[:, t * 128:t * 128 + P],
                                         ps1b[:, :P])
                    ps2 = psum.tile([H, 128], F32, tag="b")
                    nc.tensor.matmul(ps2[:, :P],
                                     convw_sb[:, li * H:(li + 1) * H],
                                     u[:, :P], start=True, stop=True)
                    xnb = pool.tile([H, 128], BF16, tag="xnb")
                    nc.scalar.activation(xnb[:, :P], ps2[:, :P],
                                         mybir.ActivationFunctionType.Relu)
                    if last:
                        psf = psum.tile([128, 128], F32, tag="b")
                        nc.tensor.matmul(psf[:C_, :P], lin1wb_sb[:],
                                         xnb[:, :P], start=True, stop=True)
                        ot = pool.tile([C_, 128], F32, tag="ot")
                        nc.vector.tensor_scalar(ot[:, :P], psf[:C_, :P],
                                                lin1b_sb[:, 0:1], None,
                                                mybir.AluOpType.add)
                        nc.sync.dma_start(out_t[:, t * 128:t * 128 + P],
                                          ot[:, :P])
                        continue
                    # next-layer stores
                    xsc = pool.tile([H, 128], F32, tag="xsc")
                    nc.scalar.activation(xsc[:, :P], ps2[:, :P],
                                         mybir.ActivationFunctionType.Relu,
                                         scale=0.45)
                    x0f = pool.tile([H, 128], F32, tag="x0f")
                    nc.scalar.dma_start(x0f[:, :P],
                                        x0sc_fm[:, t * 128:t * 128 + P])
                    xini = pool.tile([H, 128], F32, tag="xini")
                    nc.vector.tensor_add(xini[:, :P], xsc[:, :P], x0f[:, :P])
                    nc.sync.dma_start(xinit[(li + 1) % 2]
                                      [:, t * 128:t * 128 + P], xini[:, :P])
                    # node-major next x + center partial
                    psn = pstr.tile([128, 128], BF16, tag="bb")
                    nc.tensor.transpose(psn[:P, :], xnb[:, :P], identb_sb[:])
                    xnn = pool.tile([128, H], BF16, tag="xnn")
                    nc.vector.tensor_copy(xnn[:P, :], psn[:P, :])
                    nc.sync.dma_start(nxt_shp[t * 128:t * 128 + P, :],
                                      xnn[:P, :])
                    x0a = pool.tile([128, H], BF16, tag="x0a")
                    nc.scalar.dma_start(x0a[:P, :],
                                        x0s_sh[t * 128:t * 128 + P, :])
                    sN = pool.tile([128, H], F32, tag="sN")
                    nc.vector.tensor_add(sN[:P, :], xnn[:P, :], x0a[:P, :])
                    dt_ = pool.tile([128, C_], F32, tag="dt")
                    nc.scalar.dma_start(dt_[:P, :], d_t[t][:P, :])
                    nc.tensor.matmul(psA[:], dt_[:P, :], sN[:P, :],
                                     start=(t == 0), stop=(t == NT - 1))
                if not last:
                    cpn = pool.tile([C_, H], BF16, tag="cp")
                    nc.vector.tensor_copy(cpn[:], psA[:])
                    nc.sync.dma_start(nxt_shp[NS:NS + C_, :], cpn[:])
                    cpnl = pool.tile([C_, H], BF16, tag="cpl")
                    nc.vector.tensor_sub(cpnl[:], psA[:], cpn[:])
                    nc.sync.dma_start(nxt_shp[NS + C_:NS + 2 * C_, :],
                                      cpnl[:])

    nc.compile()
    return nc


# ----------------------------------------------------------------------
# host wrapper
# ----------------------------------------------------------------------

def _prep_inputs(cfg, inputs):
    c = cfg
    x = np.asarray(inputs["x"], np.float32)
    label = np.asarray(inputs["label"], np.int64)
    p = np.asarray(inputs["p"], np.float32)
    cm = np.asarray(inputs["cm"], np.float32)
    lin0_w = np.asarray(inputs["lin0_w"], np.float32)
    lin0_b = np.asarray(inputs["lin0_b"], np.float32)
    lin1_w = np.asarray(inputs["lin1_w"], np.float32)
    lin1_b = np.asarray(inputs["lin1_b"], np.float32)
    conv_w = np.asarray(inputs["conv_w"], np.float32)

    gidx, smat, sched = _prep_edges(cfg, inputs["edge_index"],
                                    inputs["edge_weight"])

    cnt = np.bincount(label, minlength=c.C).astype(np.float32)
    cnt = np.maximum(cnt, 1.0)
    cma = cm[:, 0, :] * (1.0 - c.alpha) * c.rsl      # fold 0.45 into r path
    i47 = np.eye(c.C, dtype=np.float32)
    ident = np.eye(128, dtype=np.float32)
    # fold beta mix into conv weights
    convw2 = np.stack([
        (1.0 - b) * np.eye(c.H, dtype=np.float32) + b * conv_w[i]
        for i, b in enumerate(
            float(np.log(c.theta / (i + 1) + 1.0)) for i in range(c.L))
    ])

    in_maps = []
    for ci in range(c.ncores):
        r0 = ci * c.NS
        xs = x[r0:r0 + c.NS]
        lab = label[r0:r0 + c.NS]
        ps = p[r0:r0 + c.NS]
        d_tile = np.zeros((c.NTP, c.C), np.float32)
        d_tile[np.arange(c.NS), lab] = 1.0 / cnt[lab]
        p_pad = np.zeros((c.NTP, c.C), np.float32)
        p_pad[:c.NS] = ps
        in_maps.append({
            "xin_t": np.ascontiguousarray(xs.T),
            "d_t": np.ascontiguousarray(d_tile.reshape(c.NT, 128, c.C)),
            "p_t": np.ascontiguousarray(
                p_pad.reshape(c.NT, 128, c.C).transpose(0, 2, 1)),
            "gidx": gidx[ci], "smat": smat[ci],
            "lin0w": lin0_w, "lin0b": lin0_b.reshape(-1, 1),
            "lin1w": lin1_w, "lin1b": lin1_b.reshape(-1, 1),
            "convw": convw2, "cma": cma,
            "cmat": np.ascontiguousarray(cma.T),
            "i47": i47, "ident": ident,
        })
    return in_maps, sched


_BUILT = {}


def kernel(**inputs):
    cfg = DEF
    global SCHED
    in_maps, sched = _prep_inputs(cfg, inputs)
    key = "default"
    if key not in _BUILT:
        SCHED = sched
        _BUILT[key] = build_nc(cfg)
    nc = _BUILT[key]
    res = bass_utils.run_bass_kernel_spmd(nc, in_maps,
                                          core_ids=list(range(cfg.ncores)))
    outs = [res.results[ci]["out_t"].T for ci in range(cfg.ncores)]
    return np.ascontiguousarray(np.concatenate(outs, 0))


# revision 12
# speedup vs baseline: 1.1000x; 1.1000x over previous
"""GCN2 (nn_GCN2_42331197669873) Bass kernel for 8 TRN2 NeuronCores.

v2 design (vs v1: gather+scatter_add both via SWDGE on one queue):

Graph/data parallel: nodes sharded row-wise across 8 cores (12500 each).
Per layer:
  - AllGather node features (bf16, node-major) into x_rep; each core's
    AllGather payload carries 47 extra rows: its partial class-center sums,
    so no separate AllReduce is needed (partials summed locally after AG).
  - Sparse propagate, pull form, per core over its dst shard:
      * edges sorted by (src window, dst); 4 windows of 2 core-blocks each
        keep gather indices int16-addressable.
      * SWDGE dma_gather of x_rep rows in 1024-token calls, round-robined
        over 4 SWDGE queues (4 queues ~2.9ns/token vs 8.4 on one; >1024
        tokens per call wedges the Q7).
      * dst-side segment-sum via PE matmul instead of dma_scatter_add:
        for each 128-token column, agg[:, d0:d0+128] += gcol^T @ S where
        S[t, d-d0] = 0.45*edge_weight (host-precomputed bf16, streamed from
        DRAM; identical across layers). Columns sit at fixed dst offsets
        d0 = min(s*c, NS-128) with a stride s shared by all cores (SPMD);
        tokens are greedily packed into eligible columns on host.
      * agg is SBUF-resident [H, 12544] f32, initialized by DMA from
        xinit = 0.45*x + 0.1*x0 (written tile-wise by the previous layer's
        dense pass), so u = agg + p@r directly.
  - Dense pass per dst tile: u = agg_slice + r_cls@p; out = u @ Wc' with
    Wc' = (1-beta)I + beta*conv_w (folded on host); relu; writes
    next-layer tensors (feature-major bf16 + node-major bf16 + xinit f32)
    and accumulates the class-center partial for the next AllGather.
  - Last layer fuses lin1 instead of writing next-layer tensors.

kernel(**inputs) takes FULL unsharded inputs, returns FULL [100000, 47].
"""

import numpy as np
import ml_dtypes

from concourse import bass, bacc, tile, mybir, bass_utils
from concourse import library_config
from concourse.mybir import AxisListType
import concourse.tile_sem_assignment as _tsa
from concourse import bass_isa as _bisa

# Tile round-robins Pool-engine DMAs over all DMASW sem lanes ignoring
# queue_num; mixing SWDGE queues on one lane breaks its in-order-completion
# assumption. Segregate lanes by queue: queue q -> lanes [2q, 2q+2).
_NQ = 4
_orig_assign_tick = _tsa.TileClockTick._assign_tick


def _assign_tick_qsplit(self, inst):
    if (isinstance(inst, _tsa.DMAInst)
            and inst.engine == mybir.EngineType.Pool
            and not isinstance(inst, _bisa.UserSyncedRemoteDMADescs)
            and self.swdge_sem_count >= _NQ):
        qn = getattr(inst, "queue_num", 0) or 0
        per = self.swdge_sem_count // _NQ
        if not hasattr(self, "_qrr"):
            self._qrr = {}
        r = self._qrr.get(qn, 0)
        self._qrr[qn] = r + 1
        self.next_sw_dma_idx = (qn % _NQ) * per + r % per
    return _orig_assign_tick(self, inst)


_tsa.TileClockTick._assign_tick = _assign_tick_qsplit

F32 = mybir.dt.float32
BF16 = mybir.dt.bfloat16
I16 = mybir.dt.int16


class Cfg:
    def __init__(self, N=100000, E=800000, C=47, H=128, ncores=8,
                 L=4, alpha=0.1, theta=0.5, rsl=0.5):
        self.N, self.E, self.C, self.H = N, E, C, H
        self.ncores = ncores
        self.NS = N // ncores                 # nodes per core (12500)
        self.NT = (self.NS + 127) // 128      # dst tiles per core (98)
        self.NTP = self.NT * 128              # padded shard rows (12544)
        self.B = self.NS + 100                # AG block: data + 2*47 cpart + pad
        self.nwin = 4                         # src windows (2 core-blocks each)
        self.L, self.alpha, self.theta, self.rsl = L, alpha, theta, rsl


DEF = Cfg()

# set by kernel()/test before build_nc: (stride, ncols, ncalls)
SCHED = None


# ----------------------------------------------------------------------
# host-side edge preprocessing
# ----------------------------------------------------------------------

def _greedy_cols(d_sorted, d0, cap=128):
    """Assign dst-sorted tokens to columns; col c covers [d0[c], d0[c]+128).

    Returns col ids or None if infeasible."""
    C = len(d0)
    fill = np.zeros(C, np.int64)
    cols = np.empty(len(d_sorted), np.int64)
    nxt = 0  # all cols < nxt are full
    # c_lo for each token: first column covering d
    for i, d in enumerate(d_sorted):
        c = np.searchsorted(d0, d - 127, side="left")
        c = max(c, nxt)
        while c < C and (fill[c] >= cap or d0[c] + 127 < d):
            if fill[c] >= cap and c == nxt:
                nxt += 1
            c += 1
        if c >= C or d0[c] > d:
            return None
        cols[i] = c
        fill[c] += 1
    return cols


def _prep_edges(cfg, edge_index, edge_weight):
    """Token layout for the matmul-segment-sum propagate.

    Returns (gidx [nc, nwin, ncalls, 128, 64] i16,
             smat [nc, nwin, ncols, 128, 128] bf16,
             sched (stride, ncols, ncalls))."""
    c = cfg
    src = np.asarray(edge_index[0], np.int64)
    dst = np.asarray(edge_index[1], np.int64)
    ew = (np.asarray(edge_weight, np.float32)
          * (1.0 - c.alpha) * (1.0 - c.rsl))          # fold 0.45
    nc, NS, B, nwin = c.ncores, c.NS, c.B, c.nwin
    WN = NS * 2                                       # nodes per window

    per_core = []
    for ci in range(nc):
        m = (dst >= ci * NS) & (dst < (ci + 1) * NS)
        s_c, d_c, w_c = src[m], dst[m] - ci * NS, ew[m]
        wi = s_c // WN
        order = np.lexsort((d_c, wi))
        per_core.append((s_c[order], d_c[order], w_c[order], wi[order]))

    # shared column schedule: try decreasing strides until all fit
    for stride in (64, 60, 56, 48, 40, 32):
        C = int(np.ceil((NS - 128) / stride)) + 1
        C = ((C + 7) // 8) * 8                        # whole 1024-tok calls
        d0 = np.minimum(stride * np.arange(C), NS - 128)
        all_cols = []
        ok = True
        for ci in range(nc):
            s_c, d_c, w_c, wi = per_core[ci]
            cols_ci = []
            for w in range(nwin):
                mw = wi == w
                cw = _greedy_cols(d_c[mw], d0)
                if cw is None:
                    ok = False
                    break
                cols_ci.append(cw)
            if not ok:
                break
            all_cols.append(cols_ci)
        if ok:
            break
    assert ok, "no feasible column stride"
    ncalls = C // 8

    gidx = np.zeros((nc, nwin, ncalls, 128, 64), np.int16)
    smat = np.zeros((nc, nwin, C, 128, 128), ml_dtypes.bfloat16)
    # device layouts: gidx [nwin, 128, ncalls*64]; smat [128, nwin*C*128]
    for ci in range(nc):
        s_c, d_c, w_c, wi = per_core[ci]
        for w in range(nwin):
            mw = wi == w
            s_w, d_w, w_w = s_c[mw], d_c[mw], w_c[mw]
            cols = all_cols[ci][w]
            # slot within column = arrival order (tokens are dst-sorted)
            order = np.argsort(cols, kind="stable")
            s_w, d_w, w_w, cols = s_w[order], d_w[order], w_w[order], cols[order]
            slot = np.arange(len(cols)) - np.searchsorted(cols, cols)
            # gather idx relative to window base (2w*B): core block + offset
            cc = s_w // NS
            rel = (cc - 2 * w) * B + (s_w - cc * NS)
            assert rel.min() >= 0 and rel.max() < 2 * B
            # S matrix entries
            smat[ci, w].reshape(-1)[
                (cols * 128 + slot) * 128 + (d_w - d0[cols])] = w_w
            # gather index layout: token t = (col%8)*128 + slot in call col//8
            call = cols // 8
            t = (cols % 8) * 128 + slot
            row0, col16 = t % 16, t // 16
            for g in range(8):
                gidx[ci, w, call, row0 + 16 * g, col16] = rel.astype(np.int16)
    return gidx, smat, (stride, C, ncalls, d0)


# ----------------------------------------------------------------------
# device program
# ----------------------------------------------------------------------

DBG_LI = 0


def build_nc(cfg):
    c = cfg
    nc = bacc.Bacc(None, target_bir_lowering=False, debug=False,
                   num_swdge_queues=_NQ)
    NT, NS, NTP, B, C_, H, L = c.NT, c.NS, c.NTP, c.B, c.C, c.H, c.L
    stride, NCOL, NCALL, D0 = SCHED
    nwin = c.nwin

    def dram_in(name, shape, dt=F32):
        return nc.declare_dram_parameter(name, shape, dt, isOutput=False)

    xin_t = dram_in("xin_t", [H, NS])
    d_t = dram_in("d_t", [NT, 128, C_])
    p_t = dram_in("p_t", [NT, C_, 128], BF16)
    gidx = dram_in("gidx", [nwin, 128, NCALL * 64], I16)
    smat = dram_in("smat", [128, nwin * NCOL * 128], BF16)
    lin0w = dram_in("lin0w", [H, H])
    lin0b = dram_in("lin0b", [H, 1])
    lin1w = dram_in("lin1w", [H, C_])
    lin1b = dram_in("lin1b", [C_, 1])
    convw = dram_in("convw", [L, H, H])
    cma = dram_in("cma", [C_, C_])
    cmat = dram_in("cmat", [C_, C_])
    i47 = dram_in("i47", [C_, C_])
    ident = dram_in("ident", [128, 128])
    out_t = nc.declare_dram_parameter("out_t", [C_, NS], F32, isOutput=True)

    # internal DRAM
    x_rep = nc.dram_tensor("x_rep", [c.ncores * B, H], BF16,
                           addr_space="Shared")
    x_shp = [nc.dram_tensor(f"x_shp{i}", [B, H], BF16) for i in range(2)]
    x0s_sh = nc.dram_tensor("x0s_sh", [NTP, H], BF16)      # 0.1*x0 node-major
    xinit = [nc.dram_tensor(f"xinit{i}", [H, NTP], F32) for i in range(2)]
    x0sc_fm = nc.dram_tensor("x0sc_fm", [H, NTP], F32)     # 0.1*x0 f-major

    rg = [list(range(c.ncores))]

    def tsize(t):
        return min(128, NS - t * 128)

    with tile.TileContext(nc) as tc:
        nc.gpsimd.load_library(library_config.mlp)
        with (
            tc.tile_pool(name="const", bufs=1) as cpool,
            tc.tile_pool(name="sb", bufs=3) as pool,
            tc.tile_pool(name="gt", bufs=6) as gpool,
            tc.tile_pool(name="st", bufs=6) as spool,
            tc.tile_pool(name="ps", bufs=3, space="PSUM") as psum,
            tc.tile_pool(name="pst", bufs=1, space="PSUM") as pstr,
            tc.tile_pool(name="psc", bufs=3, space="PSUM") as pscol,
            tc.tile_pool(name="psacc", bufs=1, space="PSUM") as psacc,
        ):
            # ---- resident constants ----
            lin0w_sb = cpool.tile([H, H], F32)
            nc.sync.dma_start(lin0w_sb[:], lin0w[:, :])
            lin0b_sb = cpool.tile([H, 1], F32)
            nc.sync.dma_start(lin0b_sb[:], lin0b[:, :])
            lin1w_sb = cpool.tile([H, C_], F32)
            nc.sync.dma_start(lin1w_sb[:], lin1w[:, :])
            lin1b_sb = cpool.tile([C_, 1], F32)
            nc.sync.dma_start(lin1b_sb[:], lin1b[:, :])
            convw_sb = cpool.tile([H, L * H], F32)
            for i in range(L):
                nc.sync.dma_start(convw_sb[:, i * H:(i + 1) * H], convw[i])
            convwb_sb = cpool.tile([H, L * H], BF16)
            nc.vector.tensor_copy(convwb_sb[:], convw_sb[:])
            cma_sb = cpool.tile([C_, C_], F32)
            nc.sync.dma_start(cma_sb[:], cma[:, :])
            cmat_sb = cpool.tile([C_, C_], F32)
            nc.sync.dma_start(cmat_sb[:], cmat[:, :])
            i47_sb = cpool.tile([C_, C_], F32)
            nc.sync.dma_start(i47_sb[:], i47[:, :])
            ident_sb = cpool.tile([128, 128], F32)
            nc.sync.dma_start(ident_sb[:], ident[:, :])
            identb_sb = cpool.tile([128, 128], BF16)
            nc.vector.tensor_copy(identb_sb[:], ident_sb[:])
            lin1wb_sb = cpool.tile([H, C_], BF16)
            nc.vector.tensor_copy(lin1wb_sb[:], lin1w_sb[:])
            b55 = cpool.tile([H, 1], F32)
            nc.vector.tensor_scalar(b55[:], lin0b_sb[:], 0.55, None,
                                    mybir.AluOpType.mult)
            b10 = cpool.tile([H, 1], F32)
            nc.vector.tensor_scalar(b10[:], lin0b_sb[:], 0.1, None,
                                    mybir.AluOpType.mult)
            agg = cpool.tile([H, NTP], F32)

            # ---- lin0: x0 = relu(x @ W0 + b0) ----
            psA = psacc.tile([C_, H], F32, tag="cen")
            for t in range(NT):
                P = tsize(t)
                xi = pool.tile([H, 128], F32, tag="xi")
                nc.sync.dma_start(xi[:, :P], xin_t[:, t * 128:t * 128 + P])
                ps0 = psum.tile([H, 128], F32, tag="b")
                nc.tensor.matmul(ps0[:, :P], lin0w_sb[:], xi[:, :P],
                                 start=True, stop=True)
                # feature-major stores: 0.55*x0 (agg init L0), 0.1*x0,
                # 0.45*x0 (cur bf16), plus node-major bf16 copies
                xi0 = pool.tile([H, 128], F32, tag="xi0")
                nc.scalar.activation(xi0[:, :P], ps0[:, :P],
                                     mybir.ActivationFunctionType.Relu,
                                     bias=b55[:, 0:1], scale=0.55)
                nc.sync.dma_start(xinit[0][:, t * 128:t * 128 + P],
                                  xi0[:, :P])
                x1 = pool.tile([H, 128], F32, tag="x1")
                nc.scalar.activation(x1[:, :P], ps0[:, :P],
                                     mybir.ActivationFunctionType.Relu,
                                     bias=b10[:, 0:1], scale=0.1)
                nc.sync.dma_start(x0sc_fm[:, t * 128:t * 128 + P], x1[:, :P])
                # node-major: x0 bf16 (AG payload) and 0.1*x0 bf16
                x0b = pool.tile([H, 128], BF16, tag="x0b")
                nc.scalar.activation(x0b[:, :P], ps0[:, :P],
                                     mybir.ActivationFunctionType.Relu,
                                     bias=lin0b_sb[:, 0:1])
                pst = pstr.tile([128, 128], BF16, tag="bb")
                nc.tensor.transpose(pst[:P, :], x0b[:, :P], identb_sb[:])
                x0n = pool.tile([128, H], BF16, tag="x0n")
                nc.vector.tensor_copy(x0n[:P, :], pst[:P, :])
                nc.sync.dma_start(x_shp[0][t * 128:t * 128 + P, :], x0n[:P, :])
                x0ns = pool.tile([128, H], BF16, tag="x0ns")
                nc.vector.tensor_scalar(x0ns[:P, :], x0n[:P, :], 0.1, None,
                                        mybir.AluOpType.mult)
                nc.sync.dma_start(x0s_sh[t * 128:t * 128 + P, :],
                                  x0ns[:P, :])
                # center partial for layer 0: s = 1.1 * x0
                s0 = pool.tile([128, H], F32, tag="s0")
                nc.vector.tensor_scalar(s0[:P, :], x0n[:P, :], 1.1, None,
                                        mybir.AluOpType.mult)
                dt_ = pool.tile([128, C_], F32, tag="dt")
                nc.scalar.dma_start(dt_[:P, :], d_t[t][:P, :])
                nc.tensor.matmul(psA[:], dt_[:P, :], s0[:P, :],
                                 start=(t == 0), stop=(t == NT - 1))
            cp0 = pool.tile([C_, H], BF16, tag="cp")
            nc.vector.tensor_copy(cp0[:], psA[:])
            nc.sync.dma_start(x_shp[0][NS:NS + C_, :], cp0[:])
            cp0l = pool.tile([C_, H], BF16, tag="cpl")
            nc.vector.tensor_sub(cp0l[:], psA[:], cp0[:])
            nc.sync.dma_start(x_shp[0][NS + C_:NS + 2 * C_, :], cp0l[:])

            # ---- layers ----
            qrr = 0
            for li in range(L):
                cur_shp = x_shp[li % 2]
                nxt_shp = x_shp[(li + 1) % 2]

                nc.gpsimd.collective_compute(
                    "AllGather", mybir.AluOpType.bypass, replica_groups=rg,
                    ins=[cur_shp.ap().opt()], outs=[x_rep.ap().opt()],
                )

                # agg init = 0.45*x + 0.1*x0 (f32, feature-major)
                nc.scalar.dma_start(agg[:, :], xinit[li % 2][:, :])

                # centers from AG payload (hi+lo bf16 pairs -> ~f32)
                cen = pool.tile([C_, H], F32, tag="cen_sb")
                cp_a = pool.tile([C_, H], BF16, tag="cpa")
                nc.sync.dma_start(cp_a[:], x_rep[NS:NS + C_, :])
                cp_b = pool.tile([C_, H], BF16, tag="cpb")
                nc.sync.dma_start(cp_b[:], x_rep[NS + C_:NS + 2 * C_, :])
                nc.vector.tensor_add(cen[:], cp_a[:], cp_b[:])
                for ci in range(1, c.ncores):
                    for half in range(2):
                        cp_i = pool.tile([C_, H], BF16, tag="cpi")
                        r0 = ci * B + NS + half * C_
                        nc.sync.dma_start(cp_i[:], x_rep[r0:r0 + C_, :])
                        nc.vector.tensor_add(cen[:], cen[:], cp_i[:])

                # r_cls from centers (Gram trick); cma pre-scaled by 0.45
                pst = psum.tile([128, 128], F32, tag="b")
                nc.tensor.transpose(pst[:, :C_], cen[:], ident_sb[:C_, :C_])
                cT = pool.tile([128, C_], F32, tag="cT")
                nc.vector.tensor_copy(cT[:], pst[:, :C_])
                psg = psum.tile([128, 128], F32, tag="b")
                nc.tensor.matmul(psg[:C_, :C_], cT[:], cT[:], start=True,
                                 stop=True)
                g = pool.tile([C_, C_], F32, tag="gg")
                nc.vector.tensor_copy(g[:], psg[:C_, :C_])
                gd = pool.tile([C_, C_], F32, tag="gd")
                nc.vector.tensor_mul(gd[:], g[:], i47_sb[:])
                n2 = pool.tile([C_, 1], F32, tag="n2")
                nc.vector.reduce_sum(n2[:], gd[:], AxisListType.X)
                t1 = pool.tile([C_, C_], F32, tag="t1")
                nc.vector.tensor_scalar(t1[:], g[:], -1.0, n2[:, 0:1],
                                        mybir.AluOpType.mult,
                                        mybir.AluOpType.add)
                ps1 = psum.tile([128, 128], F32, tag="b")
                nc.tensor.transpose(ps1[:C_, :C_], t1[:], ident_sb[:C_, :C_])
                nrm = pool.tile([C_, C_], F32, tag="nrm")
                nc.vector.tensor_add(nrm[:], t1[:], ps1[:C_, :C_])
                nc.vector.tensor_relu(nrm[:], nrm[:])
                nc.vector.tensor_add(nrm[:], nrm[:], i47_sb[:])
                rn = pool.tile([C_, C_], F32, tag="rn")
                nc.scalar.sqrt(rn[:], nrm[:])
                inv = pool.tile([C_, C_], F32, tag="inv")
                nc.vector.reciprocal(inv[:], rn[:])
                amat = pool.tile([C_, C_], F32, tag="amat")
                nc.vector.tensor_mul(amat[:], cma_sb[:], inv[:])
                atm = pool.tile([C_, C_], F32, tag="atm")
                nc.vector.tensor_mul(atm[:], cmat_sb[:], inv[:])
                rs = pool.tile([C_, 1], F32, tag="rs")
                nc.vector.reduce_sum(rs[:], amat[:], AxisListType.X)
                psm = psum.tile([128, 128], F32, tag="b")
                nc.tensor.matmul(psm[:C_, :], atm[:], cen[:], start=True,
                                 stop=True)
                rcls = pool.tile([C_, H], F32, tag="rcls")
                nc.vector.tensor_scalar(rcls[:], cen[:], rs[:, 0:1], None,
                                        mybir.AluOpType.mult)
                nc.vector.tensor_sub(rcls[:], rcls[:], psm[:C_, :])
                rclsb = pool.tile([C_, H], BF16, tag="rclsb")
                nc.vector.tensor_copy(rclsb[:], rcls[:])

                # - propagate: gather + segment matmul -
                last = li == L - 1

                def emit_passB(t):
                    P = tsize(t)
                    pt = pool.tile([C_, 128], BF16, tag="pt")
                    nc.sync.dma_start(pt[:], p_t[t])
                    ps1b = psum.tile([H, 128], F32, tag="b")
                    nc.tensor.matmul(ps1b[:, :P], rclsb[:], pt[:, :P],
                                     start=True, stop=True)
                    u = pool.tile([H, 128], BF16, tag="u")
                    nc.vector.tensor_add(u[:, :P],
                                         agg[:, t * 128:t * 128 + P],
                                         ps1b[:, :P])
                    ps2 = psum.tile([H, 128], F32, tag="b")
                    nc.tensor.matmul(ps2[:, :P],
                                     convwb_sb[:, li * H:(li + 1) * H],
                                     u[:, :P], start=True, stop=True)
                    xnb = pool.tile([H, 128], BF16, tag="xnb")
                    nc.scalar.activation(xnb[:, :P], ps2[:, :P],
                                         mybir.ActivationFunctionType.Relu)
                    if last:
                        psf = psum.tile([128, 128], F32, tag="b")
                        nc.tensor.matmul(psf[:C_, :P], lin1wb_sb[:],
                                         xnb[:, :P], start=True, stop=True)
                        ot = pool.tile([C_, 128], F32, tag="ot")
                        nc.vector.tensor_scalar(ot[:, :P], psf[:C_, :P],
                                                lin1b_sb[:, 0:1], None,
                                                mybir.AluOpType.add)
                        nc.sync.dma_start(out_t[:, t * 128:t * 128 + P],
                                          ot[:, :P])
                        return
                    xsc = pool.tile([H, 128], F32, tag="xsc")
                    nc.scalar.activation(xsc[:, :P], ps2[:, :P],
                                         mybir.ActivationFunctionType.Relu,
                                         scale=0.45)
                    x0f = pool.tile([H, 128], F32, tag="x0f")
                    nc.scalar.dma_start(x0f[:, :P],
                                        x0sc_fm[:, t * 128:t * 128 + P])
                    xini = pool.tile([H, 128], F32, tag="xini")
                    nc.vector.tensor_add(xini[:, :P], xsc[:, :P], x0f[:, :P])
                    nc.sync.dma_start(xinit[(li + 1) % 2]
                                      [:, t * 128:t * 128 + P], xini[:, :P])
                    psn = pstr.tile([128, 128], BF16, tag="bb")
                    nc.tensor.transpose(psn[:P, :], xnb[:, :P], identb_sb[:])
                    xnn = pool.tile([128, H], BF16, tag="xnn")
                    nc.vector.tensor_copy(xnn[:P, :], psn[:P, :])
                    nc.sync.dma_start(nxt_shp[t * 128:t * 128 + P, :],
                                      xnn[:P, :])
                    x0a = pool.tile([128, H], BF16, tag="x0a")
                    nc.scalar.dma_start(x0a[:P, :],
                                        x0s_sh[t * 128:t * 128 + P, :])
                    sN = pool.tile([128, H], F32, tag="sN")
                    nc.vector.tensor_add(sN[:P, :], xnn[:P, :], x0a[:P, :])
                    dt_ = pool.tile([128, C_], F32, tag="dt")
                    nc.scalar.dma_start(dt_[:P, :], d_t[t][:P, :])
                    nc.tensor.matmul(psA[:], dt_[:P, :], sN[:P, :],
                                     start=(t == 0), stop=(t == NT - 1))

                t_emitted = 0
                for w in range(nwin):
                    wbase = 2 * w * B
                    gi_w = spool.tile([128, NCALL * 64], I16, tag="giw")
                    nc.scalar.dma_start(gi_w[:], gidx[w])
                    for k in range(NCALL):
                        st = spool.tile([128, 8, 128], BF16, tag="st")
                        c0 = (w * NCOL + k * 8) * 128
                        nc.sync.dma_start(
                            st[:, :, :].opt(),
                            smat[:, c0:c0 + 8 * 128])
                        gt = gpool.tile([128, 8, 128], BF16, tag="g")
                        nc.gpsimd.dma_gather(
                            gt[:, :, :], x_rep[wbase:wbase + 2 * B, :],
                            gi_w[:, k * 64:(k + 1) * 64],
                            num_idxs=1024, num_idxs_reg=1024,
                            elem_size=H, queue_num=qrr % _NQ,
                        )
                        qrr += 1
                        for j in range(8):
                            col = k * 8 + j
                            d0 = int(D0[col])
                            psC = pscol.tile([H, 128], F32, tag="pc")
                            nc.tensor.matmul(psC[:, :], gt[:, j, :],
                                             st[:, j, :],
                                             start=True, stop=True)
                            nc.vector.tensor_add(agg[:, d0:d0 + 128],
                                                 agg[:, d0:d0 + 128],
                                                 psC[:, :])
                        if w == nwin - 1:
                            if k == NCALL - 1:
                                t_done = NT - 1
                            else:
                                t_done = min(stride * 8 * (k + 1),
                                             NS - 128) // 128 - 1
                            while t_emitted <= min(t_done, NT - 1):
                                emit_passB(t_emitted)
                                t_emitted += 1
                if not last:
                    cpn = pool.tile([C_, H], BF16, tag="cp")
                    nc.vector.tensor_copy(cpn[:], psA[:])
                    nc.sync.dma_start(nxt_shp[NS:NS + C_, :], cpn[:])
                    cpnl = pool.tile([C_, H], BF16, tag="cpl")
                    nc.vector.tensor_sub(cpnl[:], psA[:], cpn[:])
                    nc.sync.dma_start(nxt_shp[NS + C_:NS + 2 * C_, :],
                                      cpnl[:])

    nc.compile()
    return nc


# ----------------------------------------------------------------------
# host wrapper
# ----------------------------------------------------------------------

def _prep_inputs(cfg, inputs):
    c = cfg
    x = np.asarray(inputs["x"], np.float32)
    label = np.asarray(inputs["label"], np.int64)
    p = np.asarray(inputs["p"], np.float32)
    cm = np.asarray(inputs["cm"], np.float32)
    lin0_w = np.asarray(inputs["lin0_w"], np.float32)
    lin0_b = np.asarray(inputs["lin0_b"], np.float32)
    lin1_w = np.asarray(inputs["lin1_w"], np.float32)
    lin1_b = np.asarray(inputs["lin1_b"], np.float32)
    conv_w = np.asarray(inputs["conv_w"], np.float32)

    gidx, smat, sched = _prep_edges(cfg, inputs["edge_index"],
                                    inputs["edge_weight"])

    cnt = np.bincount(label, minlength=c.C).astype(np.float32)
    cnt = np.maximum(cnt, 1.0)
    cma = cm[:, 0, :] * (1.0 - c.alpha) * c.rsl      # fold 0.45 into r path
    i47 = np.eye(c.C, dtype=np.float32)
    ident = np.eye(128, dtype=np.float32)
    # fold beta mix into conv weights
    convw2 = np.stack([
        (1.0 - b) * np.eye(c.H, dtype=np.float32) + b * conv_w[i]
        for i, b in enumerate(
            float(np.log(c.theta / (i + 1) + 1.0)) for i in range(c.L))
    ])

    in_maps = []
    for ci in range(c.ncores):
        r0 = ci * c.NS
        xs = x[r0:r0 + c.NS]
        lab = label[r0:r0 + c.NS]
        ps = p[r0:r0 + c.NS]
        d_tile = np.zeros((c.NTP, c.C), np.float32)
        d_tile[np.arange(c.NS), lab] = 1.0 / cnt[lab]
        p_pad = np.zeros((c.NTP, c.C), np.float32)
        p_pad[:c.NS] = ps
        in_maps.append({
            "xin_t": np.ascontiguousarray(xs.T),
            "d_t": np.ascontiguousarray(d_tile.reshape(c.NT, 128, c.C)),
            "p_t": np.ascontiguousarray(
                p_pad.reshape(c.NT, 128, c.C).transpose(0, 2, 1)).astype(
                    ml_dtypes.bfloat16),
            "gidx": np.ascontiguousarray(
                gidx[ci].transpose(0, 2, 1, 3).reshape(
                    c.nwin, 128, -1)),
            "smat": np.ascontiguousarray(
                smat[ci].transpose(2, 0, 1, 3).reshape(128, -1)),
            "lin0w": lin0_w, "lin0b": lin0_b.reshape(-1, 1),
            "lin1w": lin1_w, "lin1b": lin1_b.reshape(-1, 1),
            "convw": convw2, "cma": cma,
            "cmat": np.ascontiguousarray(cma.T),
            "i47": i47, "ident": ident,
        })
    return in_maps, sched


_BUILT = {}


def kernel(**inputs):
    cfg = DEF
    global SCHED
    in_maps, sched = _prep_inputs(cfg, inputs)
    key = "default"
    if key not in _BUILT:
        SCHED = sched
        _BUILT[key] = build_nc(cfg)
    nc = _BUILT[key]
    res = bass_utils.run_bass_kernel_spmd(nc, in_maps,
                                          core_ids=list(range(cfg.ncores)))
    outs = [res.results[ci]["out_t"].T for ci in range(cfg.ncores)]
    return np.ascontiguousarray(np.concatenate(outs, 0))


# revision 14
# speedup vs baseline: 2.1081x; 1.9164x over previous
"""GCN2 (nn_GCN2_42331197669873) Bass kernel for 8 TRN2 NeuronCores.

v2 design (vs v1: gather+scatter_add both via SWDGE on one queue):

Graph/data parallel: nodes sharded row-wise across 8 cores (12500 each).
Per layer:
  - AllGather node features (bf16, node-major) into x_rep; each core's
    AllGather payload carries 47 extra rows: its partial class-center sums,
    so no separate AllReduce is needed (partials summed locally after AG).
  - Sparse propagate, pull form, per core over its dst shard:
      * edges sorted by (src window, dst); 4 windows of 2 core-blocks each
        keep gather indices int16-addressable.
      * SWDGE dma_gather of x_rep rows in 1024-token calls, round-robined
        over 4 SWDGE queues (4 queues ~2.9ns/token vs 8.4 on one; >1024
        tokens per call wedges the Q7).
      * dst-side segment-sum via PE matmul instead of dma_scatter_add:
        for each 128-token column, agg[:, d0:d0+128] += gcol^T @ S where
        S[t, d-d0] = 0.45*edge_weight (host-precomputed bf16, streamed from
        DRAM; identical across layers). Columns sit at fixed dst offsets
        d0 = min(s*c, NS-128) with a stride s shared by all cores (SPMD);
        tokens are greedily packed into eligible columns on host.
      * agg is SBUF-resident [H, 12544] f32, initialized by DMA from
        xinit = 0.45*x + 0.1*x0 (written tile-wise by the previous layer's
        dense pass), so u = agg + p@r directly.
  - Dense pass per dst tile: u = agg_slice + r_cls@p; out = u @ Wc' with
    Wc' = (1-beta)I + beta*conv_w (folded on host); relu; writes
    next-layer tensors (feature-major bf16 + node-major bf16 + xinit f32)
    and accumulates the class-center partial for the next AllGather.
  - Last layer fuses lin1 instead of writing next-layer tensors.

kernel(**inputs) takes FULL unsharded inputs, returns FULL [100000, 47].
"""

import numpy as np
import ml_dtypes

from concourse import bass, bacc, tile, mybir, bass_utils
from concourse import library_config
from concourse.mybir import AxisListType
import concourse.tile_sem_assignment as _tsa
from concourse import bass_isa as _bisa

# Tile round-robins Pool-engine DMAs over all DMASW sem lanes ignoring
# queue_num; mixing SWDGE queues on one lane breaks its in-order-completion
# assumption. Segregate lanes by queue: queue q -> lanes [2q, 2q+2).
_NQ = 4
_orig_assign_tick = _tsa.TileClockTick._assign_tick


def _assign_tick_qsplit(self, inst):
    if (isinstance(inst, _tsa.DMAInst)
            and inst.engine == mybir.EngineType.Pool
            and not isinstance(inst, _bisa.UserSyncedRemoteDMADescs)
            and self.swdge_sem_count >= _NQ):
        qn = getattr(inst, "queue_num", 0) or 0
        per = self.swdge_sem_count // _NQ
        if not hasattr(self, "_qrr"):
            self._qrr = {}
        r = self._qrr.get(qn, 0)
        self._qrr[qn] = r + 1
        self.next_sw_dma_idx = (qn % _NQ) * per + r % per
    return _orig_assign_tick(self, inst)


_tsa.TileClockTick._assign_tick = _assign_tick_qsplit

F32 = mybir.dt.float32
BF16 = mybir.dt.bfloat16
I16 = mybir.dt.int16


class Cfg:
    def __init__(self, N=100000, E=800000, C=47, H=128, ncores=8,
                 L=4, alpha=0.1, theta=0.5, rsl=0.5):
        self.N, self.E, self.C, self.H = N, E, C, H
        self.ncores = ncores
        self.NS = N // ncores                 # nodes per core (12500)
        self.NT = (self.NS + 127) // 128      # dst tiles per core (98)
        self.NTP = self.NT * 128              # padded shard rows (12544)
        self.B = self.NS + 100                # AG block: data + 2*47 cpart + pad
        self.nwin = 4                         # src windows (2 core-blocks each)
        self.L, self.alpha, self.theta, self.rsl = L, alpha, theta, rsl


DEF = Cfg()

# set by kernel()/test before build_nc: (stride, ncols, ncalls)
SCHED = None


# ----------------------------------------------------------------------
# host-side edge preprocessing
# ----------------------------------------------------------------------

def _greedy_cols(d_sorted, d0, cap=128):
    """Assign dst-sorted tokens to columns; col c covers [d0[c], d0[c]+128).

    Returns col ids or None if infeasible."""
    C = len(d0)
    fill = np.zeros(C, np.int64)
    cols = np.empty(len(d_sorted), np.int64)
    nxt = 0  # all cols < nxt are full
    # c_lo for each token: first column covering d
    for i, d in enumerate(d_sorted):
        c = np.searchsorted(d0, d - 127, side="left")
        c = max(c, nxt)
        while c < C and (fill[c] >= cap or d0[c] + 127 < d):
            if fill[c] >= cap and c == nxt:
                nxt += 1
            c += 1
        if c >= C or d0[c] > d:
            return None
        cols[i] = c
        fill[c] += 1
    return cols


def _prep_edges(cfg, edge_index, edge_weight):
    """Token layout for the matmul-segment-sum propagate.

    Returns (gidx [nc, nwin, ncalls, 128, 64] i16,
             smat [nc, nwin, ncols, 128, 128] bf16,
             sched (stride, ncols, ncalls))."""
    c = cfg
    src = np.asarray(edge_index[0], np.int64)
    dst = np.asarray(edge_index[1], np.int64)
    ew = (np.asarray(edge_weight, np.float32)
          * (1.0 - c.alpha) * (1.0 - c.rsl))          # fold 0.45
    nc, NS, B, nwin = c.ncores, c.NS, c.B, c.nwin
    WN = NS * 2                                       # nodes per window

    per_core = []
    for ci in range(nc):
        m = (dst >= ci * NS) & (dst < (ci + 1) * NS)
        s_c, d_c, w_c = src[m], dst[m] - ci * NS, ew[m]
        wi = s_c // WN
        order = np.lexsort((d_c, wi))
        per_core.append((s_c[order], d_c[order], w_c[order], wi[order]))

    # adaptive shared column schedule (joint earliest-dst sweep per window)
    d0_w, cols_all = [], [[] for _ in range(nc)]
    for w in range(nwin):
        dsts = [per_core[ci][1][per_core[ci][3] == w] for ci in range(nc)]
        ptrs = [0] * nc
        d0l = []
        while True:
            rem = [ci for ci in range(nc) if ptrs[ci] < len(dsts[ci])]
            if not rem:
                break
            a = min(int(dsts[ci][ptrs[ci]]) for ci in rem)
            a = min(a, NS - 128)
            d0l.append(a)
            for ci in rem:
                d = dsts[ci]
                hi = np.searchsorted(d, a + 128, side="left")
                ptrs[ci] = min(ptrs[ci] + 128, hi)
        ncol = ((len(d0l) + 7) // 8) * 8
        d0l += [NS - 128] * (ncol - len(d0l))
        d0a = np.asarray(d0l, np.int64)
        d0_w.append(d0a)
        for ci in range(nc):
            cw = _greedy_cols(dsts[ci], d0a)
            assert cw is not None, f"schedule infeasible w={w} core={ci}"
            cols_all[ci].append(cw)

    ncols_w = [len(d) for d in d0_w]
    ncalls_w = [n // 8 for n in ncols_w]
    col_off = np.concatenate([[0], np.cumsum(ncols_w)])[:nwin + 1]
    call_off = [c // 8 for c in col_off]
    totcol = int(col_off[-1])
    totcall = totcol // 8
    gidx = np.zeros((nc, 128, totcall * 64), np.int16)
    smat = np.zeros((nc, 128, totcol * 128), ml_dtypes.bfloat16)
    for ci in range(nc):
        s_c, d_c, w_c, wi = per_core[ci]
        for w in range(nwin):
            mw = wi == w
            s_w, d_w, w_w = s_c[mw], d_c[mw], w_c[mw]
            cols = cols_all[ci][w]
            d0a = d0_w[w]
            order = np.argsort(cols, kind="stable")
            s_w, d_w, w_w, cols = s_w[order], d_w[order], w_w[order], cols[order]
            slot = np.arange(len(cols)) - np.searchsorted(cols, cols)
            cc = s_w // NS
            rel = (cc - 2 * w) * B + (s_w - cc * NS)
            assert rel.min() >= 0 and rel.max() < 2 * B
            gcol = col_off[w] + cols
            # S: [128 slot-partitions, totcol*128]
            smat[ci].reshape(-1)[
                (slot * totcol + gcol) * 128 + (d_w - d0a[cols])] = w_w
            call = gcol // 8
            t = (gcol % 8) * 128 + slot
            row0, col16 = t % 16, t // 16
            flat = gidx[ci].reshape(-1)
            for g in range(8):
                flat[(row0 + 16 * g) * (totcall * 64) + call * 64 + col16] = \
                    rel.astype(np.int16)
    d0_flat = np.concatenate(d0_w)
    return gidx, smat, (d0_flat, ncalls_w, call_off, col_off, totcall, totcol)
    return gidx, smat, (stride, C, ncalls, d0)


# ----------------------------------------------------------------------
# device program
# ----------------------------------------------------------------------

DBG_LI = 0


def build_nc(cfg):
    c = cfg
    nc = bacc.Bacc(None, target_bir_lowering=False, debug=False,
                   num_swdge_queues=_NQ)
    NT, NS, NTP, B, C_, H, L = c.NT, c.NS, c.NTP, c.B, c.C, c.H, c.L
    D0, NCALLS_W, CALL_OFF, COL_OFF, TOTCALL, TOTCOL = SCHED
    nwin = c.nwin

    def dram_in(name, shape, dt=F32):
        return nc.declare_dram_parameter(name, shape, dt, isOutput=False)

    xin_t = dram_in("xin_t", [H, NS])
    d_t = dram_in("d_t", [NT, 128, C_])
    p_t = dram_in("p_t", [NT, C_, 128], BF16)
    gidx = dram_in("gidx", [128, TOTCALL * 64], I16)
    smat = dram_in("smat", [128, TOTCOL * 128], BF16)
    lin0w = dram_in("lin0w", [H, H])
    lin0b = dram_in("lin0b", [H, 1])
    lin1w = dram_in("lin1w", [H, C_])
    lin1b = dram_in("lin1b", [C_, 1])
    convw = dram_in("convw", [L, H, H])
    cma = dram_in("cma", [C_, C_])
    cmat = dram_in("cmat", [C_, C_])
    i47 = dram_in("i47", [C_, C_])
    ident = dram_in("ident", [128, 128])
    out_t = nc.declare_dram_parameter("out_t", [C_, NS], F32, isOutput=True)

    # internal DRAM
    x_rep = nc.dram_tensor("x_rep", [c.ncores * B, H], BF16,
                           addr_space="Shared")
    x_shp = [nc.dram_tensor(f"x_shp{i}", [B, H], BF16) for i in range(2)]
    x0s_sh = nc.dram_tensor("x0s_sh", [NTP, H], BF16)      # 0.1*x0 node-major
    xinit = [nc.dram_tensor(f"xinit{i}", [H, NTP], F32) for i in range(2)]
    x0sc_fm = nc.dram_tensor("x0sc_fm", [H, NTP], F32)     # 0.1*x0 f-major

    rg = [list(range(c.ncores))]

    def tsize(t):
        return min(128, NS - t * 128)

    with tile.TileContext(nc) as tc:
        nc.gpsimd.load_library(library_config.mlp)
        with (
            tc.tile_pool(name="const", bufs=1) as cpool,
            tc.tile_pool(name="sb", bufs=3) as pool,
            tc.tile_pool(name="gt", bufs=6) as gpool,
            tc.tile_pool(name="st", bufs=6) as spool,
            tc.tile_pool(name="ps", bufs=3, space="PSUM") as psum,
            tc.tile_pool(name="pst", bufs=1, space="PSUM") as pstr,
            tc.tile_pool(name="psc", bufs=3, space="PSUM") as pscol,
            tc.tile_pool(name="psacc", bufs=1, space="PSUM") as psacc,
        ):
            # ---- resident constants ----
            lin0w_sb = cpool.tile([H, H], F32)
            nc.sync.dma_start(lin0w_sb[:], lin0w[:, :])
            lin0b_sb = cpool.tile([H, 1], F32)
            nc.sync.dma_start(lin0b_sb[:], lin0b[:, :])
            lin1w_sb = cpool.tile([H, C_], F32)
            nc.sync.dma_start(lin1w_sb[:], lin1w[:, :])
            lin1b_sb = cpool.tile([C_, 1], F32)
            nc.sync.dma_start(lin1b_sb[:], lin1b[:, :])
            convw_sb = cpool.tile([H, L * H], F32)
            for i in range(L):
                nc.sync.dma_start(convw_sb[:, i * H:(i + 1) * H], convw[i])
            convwb_sb = cpool.tile([H, L * H], BF16)
            nc.vector.tensor_copy(convwb_sb[:], convw_sb[:])
            cma_sb = cpool.tile([C_, C_], F32)
            nc.sync.dma_start(cma_sb[:], cma[:, :])
            cmat_sb = cpool.tile([C_, C_], F32)
            nc.sync.dma_start(cmat_sb[:], cmat[:, :])
            i47_sb = cpool.tile([C_, C_], F32)
            nc.sync.dma_start(i47_sb[:], i47[:, :])
            ident_sb = cpool.tile([128, 128], F32)
            nc.sync.dma_start(ident_sb[:], ident[:, :])
            identb_sb = cpool.tile([128, 128], BF16)
            nc.vector.tensor_copy(identb_sb[:], ident_sb[:])
            lin1wb_sb = cpool.tile([H, C_], BF16)
            nc.vector.tensor_copy(lin1wb_sb[:], lin1w_sb[:])
            b55 = cpool.tile([H, 1], F32)
            nc.vector.tensor_scalar(b55[:], lin0b_sb[:], 0.55, None,
                                    mybir.AluOpType.mult)
            b10 = cpool.tile([H, 1], F32)
            nc.vector.tensor_scalar(b10[:], lin0b_sb[:], 0.1, None,
                                    mybir.AluOpType.mult)
            agg = cpool.tile([H, NTP], F32)
            gi_all = cpool.tile([128, TOTCALL * 64], I16)
            nc.sync.dma_start(gi_all[:], gidx[:, :])

            # ---- lin0: x0 = relu(x @ W0 + b0) ----
            psA = psacc.tile([C_, H], F32, tag="cen")
            for t in range(NT):
                P = tsize(t)
                xi = pool.tile([H, 128], F32, tag="xi")
                nc.sync.dma_start(xi[:, :P], xin_t[:, t * 128:t * 128 + P])
                ps0 = psum.tile([H, 128], F32, tag="b")
                nc.tensor.matmul(ps0[:, :P], lin0w_sb[:], xi[:, :P],
                                 start=True, stop=True)
                # feature-major stores: 0.55*x0 (agg init L0), 0.1*x0,
                # 0.45*x0 (cur bf16), plus node-major bf16 copies
                xi0 = pool.tile([H, 128], F32, tag="xi0")
                nc.scalar.activation(xi0[:, :P], ps0[:, :P],
                                     mybir.ActivationFunctionType.Relu,
                                     bias=b55[:, 0:1], scale=0.55)
                nc.sync.dma_start(xinit[0][:, t * 128:t * 128 + P],
                                  xi0[:, :P])
                x1 = pool.tile([H, 128], F32, tag="x1")
                nc.scalar.activation(x1[:, :P], ps0[:, :P],
                                     mybir.ActivationFunctionType.Relu,
                                     bias=b10[:, 0:1], scale=0.1)
                nc.sync.dma_start(x0sc_fm[:, t * 128:t * 128 + P], x1[:, :P])
                # node-major: x0 bf16 (AG payload) and 0.1*x0 bf16
                x0b = pool.tile([H, 128], BF16, tag="x0b")
                nc.scalar.activation(x0b[:, :P], ps0[:, :P],
                                     mybir.ActivationFunctionType.Relu,
                                     bias=lin0b_sb[:, 0:1])
                pst = pstr.tile([128, 128], BF16, tag="bb")
                nc.tensor.transpose(pst[:P, :], x0b[:, :P], identb_sb[:])
                x0n = pool.tile([128, H], BF16, tag="x0n")
                nc.vector.tensor_copy(x0n[:P, :], pst[:P, :])
                nc.sync.dma_start(x_shp[0][t * 128:t * 128 + P, :], x0n[:P, :])
                x0ns = pool.tile([128, H], BF16, tag="x0ns")
                nc.vector.tensor_scalar(x0ns[:P, :], x0n[:P, :], 0.1, None,
                                        mybir.AluOpType.mult)
                nc.sync.dma_start(x0s_sh[t * 128:t * 128 + P, :],
                                  x0ns[:P, :])
                # center partial for layer 0: s = 1.1 * x0
                s0 = pool.tile([128, H], F32, tag="s0")
                nc.vector.tensor_scalar(s0[:P, :], x0n[:P, :], 1.1, None,
                                        mybir.AluOpType.mult)
                dt_ = pool.tile([128, C_], F32, tag="dt")
                nc.scalar.dma_start(dt_[:P, :], d_t[t][:P, :])
                nc.tensor.matmul(psA[:], dt_[:P, :], s0[:P, :],
                                 start=(t == 0), stop=(t == NT - 1))
            cp0 = pool.tile([C_, H], BF16, tag="cp")
            nc.vector.tensor_copy(cp0[:], psA[:])
            nc.sync.dma_start(x_shp[0][NS:NS + C_, :], cp0[:])
            cp0l = pool.tile([C_, H], BF16, tag="cpl")
            nc.vector.tensor_sub(cp0l[:], psA[:], cp0[:])
            nc.sync.dma_start(x_shp[0][NS + C_:NS + 2 * C_, :], cp0l[:])

            # ---- layers ----
            qrr = 0
            for li in range(L):
                cur_shp = x_shp[li % 2]
                nxt_shp = x_shp[(li + 1) % 2]

                nc.gpsimd.collective_compute(
                    "AllGather", mybir.AluOpType.bypass, replica_groups=rg,
                    ins=[cur_shp.ap().opt()], outs=[x_rep.ap().opt()],
                )

                # agg init = 0.45*x + 0.1*x0 (f32, feature-major)
                nc.scalar.dma_start(agg[:, :], xinit[li % 2][:, :])

                # centers from AG payload (hi+lo bf16 pairs -> ~f32)
                cen = pool.tile([C_, H], F32, tag="cen_sb")
                cp_a = pool.tile([C_, H], BF16, tag="cpa")
                nc.sync.dma_start(cp_a[:], x_rep[NS:NS + C_, :])
                cp_b = pool.tile([C_, H], BF16, tag="cpb")
                nc.sync.dma_start(cp_b[:], x_rep[NS + C_:NS + 2 * C_, :])
                nc.vector.tensor_add(cen[:], cp_a[:], cp_b[:])
                for ci in range(1, c.ncores):
                    for half in range(2):
                        cp_i = pool.tile([C_, H], BF16, tag="cpi")
                        r0 = ci * B + NS + half * C_
                        nc.sync.dma_start(cp_i[:], x_rep[r0:r0 + C_, :])
                        nc.vector.tensor_add(cen[:], cen[:], cp_i[:])

                # r_cls from centers (Gram trick); cma pre-scaled by 0.45
                pst = psum.tile([128, 128], F32, tag="b")
                nc.tensor.transpose(pst[:, :C_], cen[:], ident_sb[:C_, :C_])
                cT = pool.tile([128, C_], F32, tag="cT")
                nc.vector.tensor_copy(cT[:], pst[:, :C_])
                psg = psum.tile([128, 128], F32, tag="b")
                nc.tensor.matmul(psg[:C_, :C_], cT[:], cT[:], start=True,
                                 stop=True)
                g = pool.tile([C_, C_], F32, tag="gg")
                nc.vector.tensor_copy(g[:], psg[:C_, :C_])
                gd = pool.tile([C_, C_], F32, tag="gd")
                nc.vector.tensor_mul(gd[:], g[:], i47_sb[:])
                n2 = pool.tile([C_, 1], F32, tag="n2")
                nc.vector.reduce_sum(n2[:], gd[:], AxisListType.X)
                t1 = pool.tile([C_, C_], F32, tag="t1")
                nc.vector.tensor_scalar(t1[:], g[:], -1.0, n2[:, 0:1],
                                        mybir.AluOpType.mult,
                                        mybir.AluOpType.add)
                ps1 = psum.tile([128, 128], F32, tag="b")
                nc.tensor.transpose(ps1[:C_, :C_], t1[:], ident_sb[:C_, :C_])
                nrm = pool.tile([C_, C_], F32, tag="nrm")
                nc.vector.tensor_add(nrm[:], t1[:], ps1[:C_, :C_])
                nc.vector.tensor_relu(nrm[:], nrm[:])
                nc.vector.tensor_add(nrm[:], nrm[:], i47_sb[:])
                rn = pool.tile([C_, C_], F32, tag="rn")
                nc.scalar.sqrt(rn[:], nrm[:])
                inv = pool.tile([C_, C_], F32, tag="inv")
                nc.vector.reciprocal(inv[:], rn[:])
                amat = pool.tile([C_, C_], F32, tag="amat")
                nc.vector.tensor_mul(amat[:], cma_sb[:], inv[:])
                atm = pool.tile([C_, C_], F32, tag="atm")
                nc.vector.tensor_mul(atm[:], cmat_sb[:], inv[:])
                rs = pool.tile([C_, 1], F32, tag="rs")
                nc.vector.reduce_sum(rs[:], amat[:], AxisListType.X)
                psm = psum.tile([128, 128], F32, tag="b")
                nc.tensor.matmul(psm[:C_, :], atm[:], cen[:], start=True,
                                 stop=True)
                rcls = pool.tile([C_, H], F32, tag="rcls")
                nc.vector.tensor_scalar(rcls[:], cen[:], rs[:, 0:1], None,
                                        mybir.AluOpType.mult)
                nc.vector.tensor_sub(rcls[:], rcls[:], psm[:C_, :])
                rclsb = pool.tile([C_, H], BF16, tag="rclsb")
                nc.vector.tensor_copy(rclsb[:], rcls[:])

                # - propagate: gather + segment matmul -
                last = li == L - 1

                def emit_passB(t):
                    P = tsize(t)
                    pt = pool.tile([C_, 128], BF16, tag="pt")
                    nc.sync.dma_start(pt[:], p_t[t])
                    ps1b = psum.tile([H, 128], F32, tag="b")
                    nc.tensor.matmul(ps1b[:, :P], rclsb[:], pt[:, :P],
                                     start=True, stop=True)
                    u = pool.tile([H, 128], BF16, tag="u")
                    nc.vector.tensor_add(u[:, :P],
                                         agg[:, t * 128:t * 128 + P],
                                         ps1b[:, :P])
                    ps2 = psum.tile([H, 128], F32, tag="b")
                    nc.tensor.matmul(ps2[:, :P],
                                     convwb_sb[:, li * H:(li + 1) * H],
                                     u[:, :P], start=True, stop=True)
                    xnb = pool.tile([H, 128], BF16, tag="xnb")
                    nc.scalar.activation(xnb[:, :P], ps2[:, :P],
                                         mybir.ActivationFunctionType.Relu)
                    if last:
                        psf = psum.tile([128, 128], F32, tag="b")
                        nc.tensor.matmul(psf[:C_, :P], lin1wb_sb[:],
                                         xnb[:, :P], start=True, stop=True)
                        ot = pool.tile([C_, 128], F32, tag="ot")
                        nc.vector.tensor_scalar(ot[:, :P], psf[:C_, :P],
                                                lin1b_sb[:, 0:1], None,
                                                mybir.AluOpType.add)
                        nc.sync.dma_start(out_t[:, t * 128:t * 128 + P],
                                          ot[:, :P])
                        return
                    xsc = pool.tile([H, 128], F32, tag="xsc")
                    nc.scalar.activation(xsc[:, :P], ps2[:, :P],
                                         mybir.ActivationFunctionType.Relu,
                                         scale=0.45)
                    x0f = pool.tile([H, 128], F32, tag="x0f")
                    nc.scalar.dma_start(x0f[:, :P],
                                        x0sc_fm[:, t * 128:t * 128 + P])
                    xini = pool.tile([H, 128], F32, tag="xini")
                    nc.vector.tensor_add(xini[:, :P], xsc[:, :P], x0f[:, :P])
                    nc.sync.dma_start(xinit[(li + 1) % 2]
                                      [:, t * 128:t * 128 + P], xini[:, :P])
                    psn = pstr.tile([128, 128], BF16, tag="bb")
                    nc.tensor.transpose(psn[:P, :], xnb[:, :P], identb_sb[:])
                    xnn = pool.tile([128, H], BF16, tag="xnn")
                    nc.vector.tensor_copy(xnn[:P, :], psn[:P, :])
                    nc.sync.dma_start(nxt_shp[t * 128:t * 128 + P, :],
                                      xnn[:P, :])
                    x0a = pool.tile([128, H], BF16, tag="x0a")
                    nc.scalar.dma_start(x0a[:P, :],
                                        x0s_sh[t * 128:t * 128 + P, :])
                    sN = pool.tile([128, H], F32, tag="sN")
                    nc.vector.tensor_add(sN[:P, :], xnn[:P, :], x0a[:P, :])
                    dt_ = pool.tile([128, C_], F32, tag="dt")
                    nc.scalar.dma_start(dt_[:P, :], d_t[t][:P, :])
                    nc.tensor.matmul(psA[:], dt_[:P, :], sN[:P, :],
                                     start=(t == 0), stop=(t == NT - 1))

                t_emitted = 0
                for w in range(nwin):
                    wbase = 2 * w * B
                    for k in range(NCALLS_W[w]):
                        cc = CALL_OFF[w] + k
                        st = spool.tile([128, 8, 128], BF16, tag="st")
                        c0 = (COL_OFF[w] + k * 8) * 128
                        nc.sync.dma_start(
                            st[:, :, :].opt(),
                            smat[:, c0:c0 + 8 * 128])
                        gt = gpool.tile([128, 8, 128], BF16, tag="g")
                        nc.gpsimd.dma_gather(
                            gt[:, :, :], x_rep[wbase:wbase + 2 * B, :],
                            gi_all[:, cc * 64:(cc + 1) * 64],
                            num_idxs=1024, num_idxs_reg=1024,
                            elem_size=H, queue_num=qrr % _NQ,
                        )
                        qrr += 1
                        for j in range(8):
                            col = COL_OFF[w] + k * 8 + j
                            d0 = int(D0[col])
                            psC = pscol.tile([H, 128], F32, tag="pc")
                            nc.tensor.matmul(psC[:, :], gt[:, j, :],
                                             st[:, j, :],
                                             start=True, stop=True)
                            nc.vector.tensor_add(agg[:, d0:d0 + 128],
                                                 agg[:, d0:d0 + 128],
                                                 psC[:, :])
                        if w == nwin - 1:
                            nxt_col = (k + 1) * 8
                            if nxt_col >= NCALLS_W[w] * 8:
                                t_done = NT - 1
                            else:
                                t_done = int(D0[COL_OFF[w] + nxt_col]) \
                                    // 128 - 1
                            while t_emitted <= min(t_done, NT - 1):
                                emit_passB(t_emitted)
                                t_emitted += 1
                if not last:
                    cpn = pool.tile([C_, H], BF16, tag="cp")
                    nc.vector.tensor_copy(cpn[:], psA[:])
                    nc.sync.dma_start(nxt_shp[NS:NS + C_, :], cpn[:])
                    cpnl = pool.tile([C_, H], BF16, tag="cpl")
                    nc.vector.tensor_sub(cpnl[:], psA[:], cpn[:])
                    nc.sync.dma_start(nxt_shp[NS + C_:NS + 2 * C_, :],
                                      cpnl[:])

    nc.compile()
    return nc


# ----------------------------------------------------------------------
# host wrapper
# ----------------------------------------------------------------------

def _prep_inputs(cfg, inputs):
    c = cfg
    x = np.asarray(inputs["x"], np.float32)
    label = np.asarray(inputs["label"], np.int64)
    p = np.asarray(inputs["p"], np.float32)
    cm = np.asarray(inputs["cm"], np.float32)
    lin0_w = np.asarray(inputs["lin0_w"], np.float32)
    lin0_b = np.asarray(inputs["lin0_b"], np.float32)
    lin1_w = np.asarray(inputs["lin1_w"], np.float32)
    lin1_b = np.asarray(inputs["lin1_b"], np.float32)
    conv_w = np.asarray(inputs["conv_w"], np.float32)

    gidx, smat, sched = _prep_edges(cfg, inputs["edge_index"],
                                    inputs["edge_weight"])

    cnt = np.bincount(label, minlength=c.C).astype(np.float32)
    cnt = np.maximum(cnt, 1.0)
    cma = cm[:, 0, :] * (1.0 - c.alpha) * c.rsl      # fold 0.45 into r path
    i47 = np.eye(c.C, dtype=np.float32)
    ident = np.eye(128, dtype=np.float32)
    # fold beta mix into conv weights
    convw2 = np.stack([
        (1.0 - b) * np.eye(c.H, dtype=np.float32) + b * conv_w[i]
        for i, b in enumerate(
            float(np.log(c.theta / (i + 1) + 1.0)) for i in range(c.L))
    ])

    in_maps = []
    for ci in range(c.ncores):
        r0 = ci * c.NS
        xs = x[r0:r0 + c.NS]
        lab = label[r0:r0 + c.NS]
        ps = p[r0:r0 + c.NS]
        d_tile = np.zeros((c.NTP, c.C), np.float32)
        d_tile[np.arange(c.NS), lab] = 1.0 / cnt[lab]
        p_pad = np.zeros((c.NTP, c.C), np.float32)
        p_pad[:c.NS] = ps
        in_maps.append({
            "xin_t": np.ascontiguousarray(xs.T),
            "d_t": np.ascontiguousarray(d_tile.reshape(c.NT, 128, c.C)),
            "p_t": np.ascontiguousarray(
                p_pad.reshape(c.NT, 128, c.C).transpose(0, 2, 1)).astype(
                    ml_dtypes.bfloat16),
            "gidx": gidx[ci], "smat": smat[ci],
            "lin0w": lin0_w, "lin0b": lin0_b.reshape(-1, 1),
            "lin1w": lin1_w, "lin1b": lin1_b.reshape(-1, 1),
            "convw": convw2, "cma": cma,
            "cmat": np.ascontiguousarray(cma.T),
            "i47": i47, "ident": ident,
        })
    return in_maps, sched


_BUILT = {}


def kernel(**inputs):
    cfg = DEF
    global SCHED
    in_maps, sched = _prep_inputs(cfg, inputs)
    key = "default"
    if key not in _BUILT:
        SCHED = sched
        _BUILT[key] = build_nc(cfg)
    nc = _BUILT[key]
    res = bass_utils.run_bass_kernel_spmd(nc, in_maps,
                                          core_ids=list(range(cfg.ncores)))
    outs = [res.results[ci]["out_t"].T for ci in range(cfg.ncores)]
    return np.ascontiguousarray(np.concatenate(outs, 0))


# revision 17
# speedup vs baseline: 2.1976x; 1.0424x over previous
"""GCN2 (nn_GCN2_42331197669873) Bass kernel for 8 TRN2 NeuronCores.

v2 design (vs v1: gather+scatter_add both via SWDGE on one queue):

Graph/data parallel: nodes sharded row-wise across 8 cores (12500 each).
Per layer:
  - AllGather node features (bf16, node-major) into x_rep; each core's
    AllGather payload carries 47 extra rows: its partial class-center sums,
    so no separate AllReduce is needed (partials summed locally after AG).
  - Sparse propagate, pull form, per core over its dst shard:
      * edges sorted by (src window, dst); 4 windows of 2 core-blocks each
        keep gather indices int16-addressable.
      * SWDGE dma_gather of x_rep rows in 1024-token calls, round-robined
        over 4 SWDGE queues (4 queues ~2.9ns/token vs 8.4 on one; >1024
        tokens per call wedges the Q7).
      * dst-side segment-sum via PE matmul instead of dma_scatter_add:
        for each 128-token column, agg[:, d0:d0+128] += gcol^T @ S where
        S[t, d-d0] = 0.45*edge_weight (host-precomputed bf16, streamed from
        DRAM; identical across layers). Columns sit at fixed dst offsets
        d0 = min(s*c, NS-128) with a stride s shared by all cores (SPMD);
        tokens are greedily packed into eligible columns on host.
      * agg is SBUF-resident [H, 12544] f32, initialized by DMA from
        xinit = 0.45*x + 0.1*x0 (written tile-wise by the previous layer's
        dense pass), so u = agg + p@r directly.
  - Dense pass per dst tile: u = agg_slice + r_cls@p; out = u @ Wc' with
    Wc' = (1-beta)I + beta*conv_w (folded on host); relu; writes
    next-layer tensors (feature-major bf16 + node-major bf16 + xinit f32)
    and accumulates the class-center partial for the next AllGather.
  - Last layer fuses lin1 instead of writing next-layer tensors.

kernel(**inputs) takes FULL unsharded inputs, returns FULL [100000, 47].
"""

import numpy as np
import ml_dtypes

from concourse import bass, bacc, tile, mybir, bass_utils
from concourse import library_config
from concourse.mybir import AxisListType
import concourse.tile_sem_assignment as _tsa
from concourse import bass_isa as _bisa

# Tile round-robins Pool-engine DMAs over all DMASW sem lanes ignoring
# queue_num; mixing SWDGE queues on one lane breaks its in-order-completion
# assumption. Segregate lanes by queue: queue q -> lanes [2q, 2q+2).
_NQ = 4
_orig_assign_tick = _tsa.TileClockTick._assign_tick


def _assign_tick_qsplit(self, inst):
    if (isinstance(inst, _tsa.DMAInst)
            and inst.engine == mybir.EngineType.Pool
            and not isinstance(inst, _bisa.UserSyncedRemoteDMADescs)
            and self.swdge_sem_count >= _NQ):
        qn = getattr(inst, "queue_num", 0) or 0
        per = self.swdge_sem_count // _NQ
        if not hasattr(self, "_qrr"):
            self._qrr = {}
        r = self._qrr.get(qn, 0)
        self._qrr[qn] = r + 1
        self.next_sw_dma_idx = (qn % _NQ) * per + r % per
    return _orig_assign_tick(self, inst)


_tsa.TileClockTick._assign_tick = _assign_tick_qsplit

F32 = mybir.dt.float32
BF16 = mybir.dt.bfloat16
I16 = mybir.dt.int16


class Cfg:
    def __init__(self, N=100000, E=800000, C=47, H=128, ncores=8,
                 L=4, alpha=0.1, theta=0.5, rsl=0.5):
        self.N, self.E, self.C, self.H = N, E, C, H
        self.ncores = ncores
        self.NS = N // ncores                 # nodes per core (12500)
        self.NT = (self.NS + 127) // 128      # dst tiles per core (98)
        self.NTP = self.NT * 128              # padded shard rows (12544)
        self.B = self.NS + 100                # AG block: data + 2*47 cpart + pad
        self.nwin = 4                         # src windows (2 core-blocks each)
        self.L, self.alpha, self.theta, self.rsl = L, alpha, theta, rsl


DEF = Cfg()

# set by kernel()/test before build_nc: (stride, ncols, ncalls)
SCHED = None


# ----------------------------------------------------------------------
# host-side edge preprocessing
# ----------------------------------------------------------------------

def _greedy_cols(d_sorted, d0, cap=128):
    """Assign dst-sorted tokens to columns; col c covers [d0[c], d0[c]+128).

    Returns col ids or None if infeasible."""
    C = len(d0)
    fill = np.zeros(C, np.int64)
    cols = np.empty(len(d_sorted), np.int64)
    nxt = 0  # all cols < nxt are full
    # c_lo for each token: first column covering d
    for i, d in enumerate(d_sorted):
        c = np.searchsorted(d0, d - 127, side="left")
        c = max(c, nxt)
        while c < C and (fill[c] >= cap or d0[c] + 127 < d):
            if fill[c] >= cap and c == nxt:
                nxt += 1
            c += 1
        if c >= C or d0[c] > d:
            return None
        cols[i] = c
        fill[c] += 1
    return cols


def _prep_edges(cfg, edge_index, edge_weight):
    """Token layout for the matmul-segment-sum propagate.

    Returns (gidx [nc, nwin, ncalls, 128, 64] i16,
             smat [nc, nwin, ncols, 128, 128] bf16,
             sched (stride, ncols, ncalls))."""
    c = cfg
    src = np.asarray(edge_index[0], np.int64)
    dst = np.asarray(edge_index[1], np.int64)
    ew = (np.asarray(edge_weight, np.float32)
          * (1.0 - c.alpha) * (1.0 - c.rsl))          # fold 0.45
    nc, NS, B, nwin = c.ncores, c.NS, c.B, c.nwin
    WN = NS * 2                                       # nodes per window

    per_core = []
    for ci in range(nc):
        m = (dst >= ci * NS) & (dst < (ci + 1) * NS)
        s_c, d_c, w_c = src[m], dst[m] - ci * NS, ew[m]
        wi = s_c // WN
        order = np.lexsort((d_c, wi))
        per_core.append((s_c[order], d_c[order], w_c[order], wi[order]))

    # adaptive shared column schedule (joint earliest-dst sweep per window)
    d0_w, cols_all = [], [[] for _ in range(nc)]
    for w in range(nwin):
        dsts = [per_core[ci][1][per_core[ci][3] == w] for ci in range(nc)]
        ptrs = [0] * nc
        d0l = []
        while True:
            rem = [ci for ci in range(nc) if ptrs[ci] < len(dsts[ci])]
            if not rem:
                break
            a = min(int(dsts[ci][ptrs[ci]]) for ci in rem)
            a = min(a, NS - 128)
            d0l.append(a)
            for ci in rem:
                d = dsts[ci]
                hi = np.searchsorted(d, a + 128, side="left")
                ptrs[ci] = min(ptrs[ci] + 128, hi)
        ncol = ((len(d0l) + 7) // 8) * 8
        d0l += [NS - 128] * (ncol - len(d0l))
        d0a = np.asarray(d0l, np.int64)
        d0_w.append(d0a)
        for ci in range(nc):
            cw = _greedy_cols(dsts[ci], d0a)
            assert cw is not None, f"schedule infeasible w={w} core={ci}"
            cols_all[ci].append(cw)

    ncols_w = [len(d) for d in d0_w]
    ncalls_w = [n // 8 for n in ncols_w]
    col_off = np.concatenate([[0], np.cumsum(ncols_w)])[:nwin + 1]
    call_off = [c // 8 for c in col_off]
    totcol = int(col_off[-1])
    totcall = totcol // 8
    gidx = np.zeros((nc, 128, totcall * 64), np.int16)
    smat = np.zeros((nc, 128, totcol * 128), ml_dtypes.bfloat16)
    for ci in range(nc):
        s_c, d_c, w_c, wi = per_core[ci]
        for w in range(nwin):
            mw = wi == w
            s_w, d_w, w_w = s_c[mw], d_c[mw], w_c[mw]
            cols = cols_all[ci][w]
            d0a = d0_w[w]
            order = np.argsort(cols, kind="stable")
            s_w, d_w, w_w, cols = s_w[order], d_w[order], w_w[order], cols[order]
            slot = np.arange(len(cols)) - np.searchsorted(cols, cols)
            cc = s_w // NS
            rel = (cc - 2 * w) * B + (s_w - cc * NS)
            assert rel.min() >= 0 and rel.max() < 2 * B
            gcol = col_off[w] + cols
            # S: [128 slot-partitions, totcol*128]
            smat[ci].reshape(-1)[
                (slot * totcol + gcol) * 128 + (d_w - d0a[cols])] = w_w
            call = gcol // 8
            t = (gcol % 8) * 128 + slot
            row0, col16 = t % 16, t // 16
            flat = gidx[ci].reshape(-1)
            for g in range(8):
                flat[(row0 + 16 * g) * (totcall * 64) + call * 64 + col16] = \
                    rel.astype(np.int16)
    d0_flat = np.concatenate(d0_w)
    return gidx, smat, (d0_flat, ncalls_w, call_off, col_off, totcall, totcol)
    return gidx, smat, (stride, C, ncalls, d0)


# ----------------------------------------------------------------------
# device program
# ----------------------------------------------------------------------

DBG_LI = 0


def build_nc(cfg):
    c = cfg
    nc = bacc.Bacc(None, target_bir_lowering=False, debug=False,
                   num_swdge_queues=_NQ)
    NT, NS, NTP, B, C_, H, L = c.NT, c.NS, c.NTP, c.B, c.C, c.H, c.L
    D0, NCALLS_W, CALL_OFF, COL_OFF, TOTCALL, TOTCOL = SCHED
    nwin = c.nwin

    def dram_in(name, shape, dt=F32):
        return nc.declare_dram_parameter(name, shape, dt, isOutput=False)

    xin_t = dram_in("xin_t", [H, NS])
    d_t = dram_in("d_t", [NT, 128, C_])
    p_t = dram_in("p_t", [NT, C_, 128], BF16)
    gidx = dram_in("gidx", [128, TOTCALL * 64], I16)
    smat = dram_in("smat", [128, TOTCOL * 128], BF16)
    lin0w = dram_in("lin0w", [H, H])
    lin0b = dram_in("lin0b", [H, 1])
    lin1w = dram_in("lin1w", [H, C_])
    lin1b = dram_in("lin1b", [C_, 1])
    convw = dram_in("convw", [L, H, H])
    cma = dram_in("cma", [C_, C_])
    cmat = dram_in("cmat", [C_, C_])
    i47 = dram_in("i47", [C_, C_])
    ident = dram_in("ident", [128, 128])
    out_t = nc.declare_dram_parameter("out_t", [C_, NS], F32, isOutput=True)

    # internal DRAM
    x_rep = nc.dram_tensor("x_rep", [c.ncores * B, H], BF16,
                           addr_space="Shared")
    x_shp = [nc.dram_tensor(f"x_shp{i}", [B, H], BF16) for i in range(2)]
    x0s_sh = nc.dram_tensor("x0s_sh", [NTP, H], BF16)      # 0.1*x0 node-major
    xinit = [nc.dram_tensor(f"xinit{i}", [H, NTP], F32) for i in range(2)]
    x0sc_fm = nc.dram_tensor("x0sc_fm", [H, NTP], F32)     # 0.1*x0 f-major

    rg = [list(range(c.ncores))]

    def tsize(t):
        return min(128, NS - t * 128)

    with tile.TileContext(nc) as tc:
        nc.gpsimd.load_library(library_config.mlp)
        with (
            tc.tile_pool(name="const", bufs=1) as cpool,
            tc.tile_pool(name="sb", bufs=3) as pool,
            tc.tile_pool(name="gt", bufs=6) as gpool,
            tc.tile_pool(name="st", bufs=6) as spool,
            tc.tile_pool(name="ps", bufs=3, space="PSUM") as psum,
            tc.tile_pool(name="pst", bufs=1, space="PSUM") as pstr,
            tc.tile_pool(name="psc", bufs=3, space="PSUM") as pscol,
            tc.tile_pool(name="psacc", bufs=1, space="PSUM") as psacc,
        ):
            # ---- resident constants ----
            lin0w_sb = cpool.tile([H, H], F32)
            nc.sync.dma_start(lin0w_sb[:], lin0w[:, :])
            lin0b_sb = cpool.tile([H, 1], F32)
            nc.sync.dma_start(lin0b_sb[:], lin0b[:, :])
            lin1w_sb = cpool.tile([H, C_], F32)
            nc.sync.dma_start(lin1w_sb[:], lin1w[:, :])
            lin1b_sb = cpool.tile([C_, 1], F32)
            nc.sync.dma_start(lin1b_sb[:], lin1b[:, :])
            convw_sb = cpool.tile([H, L * H], F32)
            for i in range(L):
                nc.sync.dma_start(convw_sb[:, i * H:(i + 1) * H], convw[i])
            convwb_sb = cpool.tile([H, L * H], BF16)
            nc.vector.tensor_copy(convwb_sb[:], convw_sb[:])
            cma_sb = cpool.tile([C_, C_], F32)
            nc.sync.dma_start(cma_sb[:], cma[:, :])
            cmat_sb = cpool.tile([C_, C_], F32)
            nc.sync.dma_start(cmat_sb[:], cmat[:, :])
            i47_sb = cpool.tile([C_, C_], F32)
            nc.sync.dma_start(i47_sb[:], i47[:, :])
            ident_sb = cpool.tile([128, 128], F32)
            nc.sync.dma_start(ident_sb[:], ident[:, :])
            identb_sb = cpool.tile([128, 128], BF16)
            nc.vector.tensor_copy(identb_sb[:], ident_sb[:])
            lin1wb_sb = cpool.tile([H, C_], BF16)
            nc.vector.tensor_copy(lin1wb_sb[:], lin1w_sb[:])
            b55 = cpool.tile([H, 1], F32)
            nc.vector.tensor_scalar(b55[:], lin0b_sb[:], 0.55, None,
                                    mybir.AluOpType.mult)
            b10 = cpool.tile([H, 1], F32)
            nc.vector.tensor_scalar(b10[:], lin0b_sb[:], 0.1, None,
                                    mybir.AluOpType.mult)
            agg = cpool.tile([H, NTP], F32)
            gi_all = cpool.tile([128, TOTCALL * 64], I16)
            nc.sync.dma_start(gi_all[:], gidx[:, :])

            # ---- lin0: x0 = relu(x @ W0 + b0), double-tile ----
            psA = psacc.tile([C_, H], F32, tag="cen")
            NP = (NT + 1) // 2
            for dt2 in range(NP):
                r0 = dt2 * 256
                W = min(256, NS - r0)
                xi = pool.tile([H, 256], F32, tag="xi")
                nc.sync.dma_start(xi[:, :W], xin_t[:, r0:r0 + W])
                ps0 = psum.tile([H, 256], F32, tag="b")
                nc.tensor.matmul(ps0[:, :W], lin0w_sb[:], xi[:, :W],
                                 start=True, stop=True)
                xi0 = pool.tile([H, 256], F32, tag="xi0")
                nc.scalar.activation(xi0[:, :W], ps0[:, :W],
                                     mybir.ActivationFunctionType.Relu,
                                     bias=b55[:, 0:1], scale=0.55)
                nc.sync.dma_start(xinit[0][:, r0:r0 + W], xi0[:, :W])
                x1 = pool.tile([H, 256], F32, tag="x1")
                nc.scalar.activation(x1[:, :W], ps0[:, :W],
                                     mybir.ActivationFunctionType.Relu,
                                     bias=b10[:, 0:1], scale=0.1)
                nc.sync.dma_start(x0sc_fm[:, r0:r0 + W], x1[:, :W])
                x0b = pool.tile([H, 256], BF16, tag="x0b")
                nc.scalar.activation(x0b[:, :W], ps0[:, :W],
                                     mybir.ActivationFunctionType.Relu,
                                     bias=lin0b_sb[:, 0:1])
                x0n2 = pool.tile([128, 2, H], BF16, tag="x0n")
                nsub = (W + 127) // 128
                for i in range(nsub):
                    P = min(128, W - i * 128)
                    pst = pstr.tile([128, 128], BF16, tag="bb")
                    nc.tensor.transpose(pst[:P, :],
                                        x0b[:, i * 128:i * 128 + P],
                                        identb_sb[:])
                    nc.vector.tensor_copy(x0n2[:P, i, :], pst[:P, :])
                # node-major stores (row = r0 + 128*i + p)
                shp_ap = x_shp[0][r0:r0 + 256, :].rearrange(
                    "(i p) h -> p i h", i=2)
                nc.sync.dma_start(shp_ap, x0n2[:, :, :]) if W == 256 else \
                    nc.sync.dma_start(x_shp[0][r0:r0 + W, :],
                                      x0n2[:, 0, :][:W - 0, :]) \
                    if W <= 128 else [
                        nc.sync.dma_start(x_shp[0][r0:r0 + 128, :],
                                          x0n2[:, 0, :]),
                        nc.sync.dma_start(x_shp[0][r0 + 128:r0 + W, :],
                                          x0n2[:W - 128, 1, :])]
                x0ns = pool.tile([128, 2, H], BF16, tag="x0ns")
                nc.vector.tensor_scalar(x0ns[:, :, :], x0n2[:, :, :], 0.1,
                                        None, mybir.AluOpType.mult)
                x0s_ap = x0s_sh[r0:r0 + 256, :].rearrange(
                    "(i p) h -> p i h", i=2)
                if W == 256:
                    nc.sync.dma_start(x0s_ap, x0ns[:, :, :])
                else:
                    nc.sync.dma_start(x0s_sh[r0:r0 + min(W, 128), :],
                                      x0ns[:min(W, 128), 0, :])
                    if W > 128:
                        nc.sync.dma_start(x0s_sh[r0 + 128:r0 + W, :],
                                          x0ns[:W - 128, 1, :])
                s0 = pool.tile([128, 2, H], F32, tag="s0")
                nc.vector.tensor_scalar(s0[:, :, :], x0n2[:, :, :], 1.1,
                                        None, mybir.AluOpType.mult)
                dtp = pool.tile([128, 2, C_], F32, tag="dt0")
                nc.scalar.dma_start(
                    dtp[:, :nsub, :],
                    d_t[2 * dt2:2 * dt2 + nsub].rearrange(
                        "i p c -> p i c"))
                for i in range(nsub):
                    P = min(128, W - i * 128)
                    t = 2 * dt2 + i
                    nc.tensor.matmul(psA[:], dtp[:P, i, :], s0[:P, i, :],
                                     start=(t == 0), stop=(t == NT - 1))
            cp0 = pool.tile([C_, H], BF16, tag="cp")
            nc.vector.tensor_copy(cp0[:], psA[:])
            nc.sync.dma_start(x_shp[0][NS:NS + C_, :], cp0[:])
            cp0l = pool.tile([C_, H], BF16, tag="cpl")
            nc.vector.tensor_sub(cp0l[:], psA[:], cp0[:])
            nc.sync.dma_start(x_shp[0][NS + C_:NS + 2 * C_, :], cp0l[:])

            # ---- layers ----
            qrr = 0
            for li in range(L):
                cur_shp = x_shp[li % 2]
                nxt_shp = x_shp[(li + 1) % 2]

                nc.gpsimd.collective_compute(
                    "AllGather", mybir.AluOpType.bypass, replica_groups=rg,
                    ins=[cur_shp.ap().opt()], outs=[x_rep.ap().opt()],
                )

                # agg init = 0.45*x + 0.1*x0 (f32, feature-major)
                nc.scalar.dma_start(agg[:, :], xinit[li % 2][:, :])

                # centers from AG payload (hi+lo bf16 pairs -> ~f32)
                cen = pool.tile([C_, H], F32, tag="cen_sb")
                cp_a = pool.tile([C_, H], BF16, tag="cpa")
                nc.sync.dma_start(cp_a[:], x_rep[NS:NS + C_, :])
                cp_b = pool.tile([C_, H], BF16, tag="cpb")
                nc.sync.dma_start(cp_b[:], x_rep[NS + C_:NS + 2 * C_, :])
                nc.vector.tensor_add(cen[:], cp_a[:], cp_b[:])
                for ci in range(1, c.ncores):
                    for half in range(2):
                        cp_i = pool.tile([C_, H], BF16, tag="cpi")
                        r0 = ci * B + NS + half * C_
                        nc.sync.dma_start(cp_i[:], x_rep[r0:r0 + C_, :])
                        nc.vector.tensor_add(cen[:], cen[:], cp_i[:])

                # r_cls from centers (Gram trick); cma pre-scaled by 0.45
                pst = psum.tile([128, 256], F32, tag="b")
                nc.tensor.transpose(pst[:, :C_], cen[:], ident_sb[:C_, :C_])
                cT = pool.tile([128, C_], F32, tag="cT")
                nc.vector.tensor_copy(cT[:], pst[:, :C_])
                psg = psum.tile([128, 256], F32, tag="b")
                nc.tensor.matmul(psg[:C_, :C_], cT[:], cT[:], start=True,
                                 stop=True)
                g = pool.tile([C_, C_], F32, tag="gg")
                nc.vector.tensor_copy(g[:], psg[:C_, :C_])
                gd = pool.tile([C_, C_], F32, tag="gd")
                nc.vector.tensor_mul(gd[:], g[:], i47_sb[:])
                n2 = pool.tile([C_, 1], F32, tag="n2")
                nc.vector.reduce_sum(n2[:], gd[:], AxisListType.X)
                t1 = pool.tile([C_, C_], F32, tag="t1")
                nc.vector.tensor_scalar(t1[:], g[:], -1.0, n2[:, 0:1],
                                        mybir.AluOpType.mult,
                                        mybir.AluOpType.add)
                ps1 = psum.tile([128, 256], F32, tag="b")
                nc.tensor.transpose(ps1[:C_, :C_], t1[:], ident_sb[:C_, :C_])
                nrm = pool.tile([C_, C_], F32, tag="nrm")
                nc.vector.tensor_add(nrm[:], t1[:], ps1[:C_, :C_])
                nc.vector.tensor_relu(nrm[:], nrm[:])
                nc.vector.tensor_add(nrm[:], nrm[:], i47_sb[:])
                rn = pool.tile([C_, C_], F32, tag="rn")
                nc.scalar.sqrt(rn[:], nrm[:])
                inv = pool.tile([C_, C_], F32, tag="inv")
                nc.vector.reciprocal(inv[:], rn[:])
                amat = pool.tile([C_, C_], F32, tag="amat")
                nc.vector.tensor_mul(amat[:], cma_sb[:], inv[:])
                atm = pool.tile([C_, C_], F32, tag="atm")
                nc.vector.tensor_mul(atm[:], cmat_sb[:], inv[:])
                rs = pool.tile([C_, 1], F32, tag="rs")
                nc.vector.reduce_sum(rs[:], amat[:], AxisListType.X)
                psm = psum.tile([128, 256], F32, tag="b")
                nc.tensor.matmul(psm[:C_, :H], atm[:], cen[:], start=True,
                                 stop=True)
                rcls = pool.tile([C_, H], F32, tag="rcls")
                nc.vector.tensor_scalar(rcls[:], cen[:], rs[:, 0:1], None,
                                        mybir.AluOpType.mult)
                nc.vector.tensor_sub(rcls[:], rcls[:], psm[:C_, :H])
                rclsb = pool.tile([C_, H], BF16, tag="rclsb")
                nc.vector.tensor_copy(rclsb[:], rcls[:])

                # - propagate: gather + segment matmul -
                last = li == L - 1

                def emit_passB(t):
                    P = tsize(t)
                    pt = pool.tile([C_, 128], BF16, tag="pt")
                    nc.sync.dma_start(pt[:], p_t[t])
                    ps1b = psum.tile([H, 256], F32, tag="b")
                    nc.tensor.matmul(ps1b[:, :P], rclsb[:], pt[:, :P],
                                     start=True, stop=True)
                    u = pool.tile([H, 128], BF16, tag="u")
                    nc.vector.tensor_add(u[:, :P],
                                         agg[:, t * 128:t * 128 + P],
                                         ps1b[:, :P])
                    ps2 = psum.tile([H, 256], F32, tag="b")
                    nc.tensor.matmul(ps2[:, :P],
                                     convwb_sb[:, li * H:(li + 1) * H],
                                     u[:, :P], start=True, stop=True)
                    xnb = pool.tile([H, 128], BF16, tag="xnb")
                    nc.scalar.activation(xnb[:, :P], ps2[:, :P],
                                         mybir.ActivationFunctionType.Relu)
                    if last:
                        psf = psum.tile([128, 256], F32, tag="b")
                        nc.tensor.matmul(psf[:C_, :P], lin1wb_sb[:],
                                         xnb[:, :P], start=True, stop=True)
                        ot = pool.tile([C_, 128], F32, tag="ot")
                        nc.vector.tensor_scalar(ot[:, :P], psf[:C_, :P],
                                                lin1b_sb[:, 0:1], None,
                                                mybir.AluOpType.add)
                        nc.sync.dma_start(out_t[:, t * 128:t * 128 + P],
                                          ot[:, :P])
                        return
                    xsc = pool.tile([H, 128], F32, tag="xsc")
                    nc.scalar.activation(xsc[:, :P], ps2[:, :P],
                                         mybir.ActivationFunctionType.Relu,
                                         scale=0.45)
                    x0f = pool.tile([H, 128], F32, tag="x0f")
                    nc.scalar.dma_start(x0f[:, :P],
                                        x0sc_fm[:, t * 128:t * 128 + P])
                    xini = pool.tile([H, 128], F32, tag="xini")
                    nc.vector.tensor_add(xini[:, :P], xsc[:, :P], x0f[:, :P])
                    nc.sync.dma_start(xinit[(li + 1) % 2]
                                      [:, t * 128:t * 128 + P], xini[:, :P])
                    psn = pstr.tile([128, 128], BF16, tag="bb")
                    nc.tensor.transpose(psn[:P, :], xnb[:, :P], identb_sb[:])
                    xnn = pool.tile([128, H], BF16, tag="xnn")
                    nc.vector.tensor_copy(xnn[:P, :], psn[:P, :])
                    nc.sync.dma_start(nxt_shp[t * 128:t * 128 + P, :],
                                      xnn[:P, :])
                    x0a = pool.tile([128, H], BF16, tag="x0a")
                    nc.scalar.dma_start(x0a[:P, :],
                                        x0s_sh[t * 128:t * 128 + P, :])
                    sN = pool.tile([128, H], F32, tag="sN")
                    nc.vector.tensor_add(sN[:P, :], xnn[:P, :], x0a[:P, :])
                    dt_ = pool.tile([128, C_], F32, tag="dt")
                    nc.scalar.dma_start(dt_[:P, :], d_t[t][:P, :])
                    nc.tensor.matmul(psA[:], dt_[:P, :], sN[:P, :],
                                     start=(t == 0), stop=(t == NT - 1))

                t_emitted = 0
                for w in range(nwin):
                    wbase = 2 * w * B
                    for k in range(NCALLS_W[w]):
                        cc = CALL_OFF[w] + k
                        st = spool.tile([128, 8, 128], BF16, tag="st")
                        c0 = (COL_OFF[w] + k * 8) * 128
                        nc.sync.dma_start(
                            st[:, :, :].opt(),
                            smat[:, c0:c0 + 8 * 128])
                        gt = gpool.tile([128, 8, 128], BF16, tag="g")
                        nc.gpsimd.dma_gather(
                            gt[:, :, :], x_rep[wbase:wbase + 2 * B, :],
                            gi_all[:, cc * 64:(cc + 1) * 64],
                            num_idxs=1024, num_idxs_reg=1024,
                            elem_size=H, queue_num=qrr % _NQ,
                        )
                        qrr += 1
                        for j in range(8):
                            col = COL_OFF[w] + k * 8 + j
                            d0 = int(D0[col])
                            psC = pscol.tile([H, 128], F32, tag="pc")
                            nc.tensor.matmul(psC[:, :], gt[:, j, :],
                                             st[:, j, :],
                                             start=True, stop=True)
                            nc.vector.tensor_add(agg[:, d0:d0 + 128],
                                                 agg[:, d0:d0 + 128],
                                                 psC[:, :])
                        if w == nwin - 1:
                            nxt_col = (k + 1) * 8
                            if nxt_col >= NCALLS_W[w] * 8:
                                t_done = NT - 1
                            else:
                                t_done = int(D0[COL_OFF[w] + nxt_col]) \
                                    // 128 - 1
                            while t_emitted <= min(t_done, NT - 1):
                                emit_passB(t_emitted)
                                t_emitted += 1
                if not last:
                    cpn = pool.tile([C_, H], BF16, tag="cp")
                    nc.vector.tensor_copy(cpn[:], psA[:])
                    nc.sync.dma_start(nxt_shp[NS:NS + C_, :], cpn[:])
                    cpnl = pool.tile([C_, H], BF16, tag="cpl")
                    nc.vector.tensor_sub(cpnl[:], psA[:], cpn[:])
                    nc.sync.dma_start(nxt_shp[NS + C_:NS + 2 * C_, :],
                                      cpnl[:])

    nc.compile()
    return nc


# ----------------------------------------------------------------------
# host wrapper
# ----------------------------------------------------------------------

def _prep_inputs(cfg, inputs):
    c = cfg
    x = np.asarray(inputs["x"], np.float32)
    label = np.asarray(inputs["label"], np.int64)
    p = np.asarray(inputs["p"], np.float32)
    cm = np.asarray(inputs["cm"], np.float32)
    lin0_w = np.asarray(inputs["lin0_w"], np.float32)
    lin0_b = np.asarray(inputs["lin0_b"], np.float32)
    lin1_w = np.asarray(inputs["lin1_w"], np.float32)
    lin1_b = np.asarray(inputs["lin1_b"], np.float32)
    conv_w = np.asarray(inputs["conv_w"], np.float32)

    gidx, smat, sched = _prep_edges(cfg, inputs["edge_index"],
                                    inputs["edge_weight"])

    cnt = np.bincount(label, minlength=c.C).astype(np.float32)
    cnt = np.maximum(cnt, 1.0)
    cma = cm[:, 0, :] * (1.0 - c.alpha) * c.rsl      # fold 0.45 into r path
    i47 = np.eye(c.C, dtype=np.float32)
    ident = np.eye(128, dtype=np.float32)
    # fold beta mix into conv weights
    convw2 = np.stack([
        (1.0 - b) * np.eye(c.H, dtype=np.float32) + b * conv_w[i]
        for i, b in enumerate(
            float(np.log(c.theta / (i + 1) + 1.0)) for i in range(c.L))
    ])

    in_maps = []
    for ci in range(c.ncores):
        r0 = ci * c.NS
        xs = x[r0:r0 + c.NS]
        lab = label[r0:r0 + c.NS]
        ps = p[r0:r0 + c.NS]
        d_tile = np.zeros((c.NTP, c.C), np.float32)
        d_tile[np.arange(c.NS), lab] = 1.0 / cnt[lab]
        p_pad = np.zeros((c.NTP, c.C), np.float32)
        p_pad[:c.NS] = ps
        in_maps.append({
            "xin_t": np.ascontiguousarray(xs.T),
            "d_t": np.ascontiguousarray(d_tile.reshape(c.NT, 128, c.C)),
            "p_t": np.ascontiguousarray(
                p_pad.reshape(c.NT, 128, c.C).transpose(0, 2, 1)).astype(
                    ml_dtypes.bfloat16),
            "gidx": gidx[ci], "smat": smat[ci],
            "lin0w": lin0_w, "lin0b": lin0_b.reshape(-1, 1),
            "lin1w": lin1_w, "lin1b": lin1_b.reshape(-1, 1),
            "convw": convw2, "cma": cma,
            "cmat": np.ascontiguousarray(cma.T),
            "i47": i47, "ident": ident,
        })
    return in_maps, sched


_BUILT = {}


def kernel(**inputs):
    cfg = DEF
    global SCHED
    in_maps, sched = _prep_inputs(cfg, inputs)
    key = "default"
    if key not in _BUILT:
        SCHED = sched
        _BUILT[key] = build_nc(cfg)
    nc = _BUILT[key]
    res = bass_utils.run_bass_kernel_spmd(nc, in_maps,
                                          core_ids=list(range(cfg.ncores)))
    outs = [res.results[ci]["out_t"].T for ci in range(cfg.ncores)]
    return np.ascontiguousarray(np.concatenate(outs, 0))


# revision 18
# speedup vs baseline: 2.2188x; 1.0097x over previous
"""GCN2 (nn_GCN2_42331197669873) Bass kernel for 8 TRN2 NeuronCores.

v2 design (vs v1: gather+scatter_add both via SWDGE on one queue):

Graph/data parallel: nodes sharded row-wise across 8 cores (12500 each).
Per layer:
  - AllGather node features (bf16, node-major) into x_rep; each core's
    AllGather payload carries 47 extra rows: its partial class-center sums,
    so no separate AllReduce is needed (partials summed locally after AG).
  - Sparse propagate, pull form, per core over its dst shard:
      * edges sorted by (src window, dst); 4 windows of 2 core-blocks each
        keep gather indices int16-addressable.
      * SWDGE dma_gather of x_rep rows in 1024-token calls, round-robined
        over 4 SWDGE queues (4 queues ~2.9ns/token vs 8.4 on one; >1024
        tokens per call wedges the Q7).
      * dst-side segment-sum via PE matmul instead of dma_scatter_add:
        for each 128-token column, agg[:, d0:d0+128] += gcol^T @ S where
        S[t, d-d0] = 0.45*edge_weight (host-precomputed bf16, streamed from
        DRAM; identical across layers). Columns sit at fixed dst offsets
        d0 = min(s*c, NS-128) with a stride s shared by all cores (SPMD);
        tokens are greedily packed into eligible columns on host.
      * agg is SBUF-resident [H, 12544] f32, initialized by DMA from
        xinit = 0.45*x + 0.1*x0 (written tile-wise by the previous layer's
        dense pass), so u = agg + p@r directly.
  - Dense pass per dst tile: u = agg_slice + r_cls@p; out = u @ Wc' with
    Wc' = (1-beta)I + beta*conv_w (folded on host); relu; writes
    next-layer tensors (feature-major bf16 + node-major bf16 + xinit f32)
    and accumulates the class-center partial for the next AllGather.
  - Last layer fuses lin1 instead of writing next-layer tensors.

kernel(**inputs) takes FULL unsharded inputs, returns FULL [100000, 47].
"""

import numpy as np
import ml_dtypes

from concourse import bass, bacc, tile, mybir, bass_utils
from concourse import library_config
from concourse.mybir import AxisListType
import concourse.tile_sem_assignment as _tsa
from concourse import bass_isa as _bisa

# Tile round-robins Pool-engine DMAs over all DMASW sem lanes ignoring
# queue_num; mixing SWDGE queues on one lane breaks its in-order-completion
# assumption. Segregate lanes by queue: queue q -> lanes [2q, 2q+2).
_NQ = 4
_orig_assign_tick = _tsa.TileClockTick._assign_tick


def _assign_tick_qsplit(self, inst):
    if (isinstance(inst, _tsa.DMAInst)
            and inst.engine == mybir.EngineType.Pool
            and not isinstance(inst, _bisa.UserSyncedRemoteDMADescs)
            and self.swdge_sem_count >= _NQ):
        qn = getattr(inst, "queue_num", 0) or 0
        per = self.swdge_sem_count // _NQ
        if not hasattr(self, "_qrr"):
            self._qrr = {}
        r = self._qrr.get(qn, 0)
        self._qrr[qn] = r + 1
        self.next_sw_dma_idx = (qn % _NQ) * per + r % per
    return _orig_assign_tick(self, inst)


_tsa.TileClockTick._assign_tick = _assign_tick_qsplit

F32 = mybir.dt.float32
BF16 = mybir.dt.bfloat16
I16 = mybir.dt.int16


class Cfg:
    def __init__(self, N=100000, E=800000, C=47, H=128, ncores=8,
                 L=4, alpha=0.1, theta=0.5, rsl=0.5):
        self.N, self.E, self.C, self.H = N, E, C, H
        self.ncores = ncores
        self.NS = N // ncores                 # nodes per core (12500)
        self.NT = (self.NS + 127) // 128      # dst tiles per core (98)
        self.NTP = self.NT * 128              # padded shard rows (12544)
        self.B = self.NS + 100                # AG block: data + 2*47 cpart + pad
        self.nwin = 4                         # src windows (2 core-blocks each)
        self.L, self.alpha, self.theta, self.rsl = L, alpha, theta, rsl


DEF = Cfg()

# set by kernel()/test before build_nc: (stride, ncols, ncalls)
SCHED = None


# ----------------------------------------------------------------------
# host-side edge preprocessing
# ----------------------------------------------------------------------

def _greedy_cols(d_sorted, d0, cap=128):
    """Assign dst-sorted tokens to columns; col c covers [d0[c], d0[c]+128).

    Returns col ids or None if infeasible."""
    C = len(d0)
    fill = np.zeros(C, np.int64)
    cols = np.empty(len(d_sorted), np.int64)
    nxt = 0  # all cols < nxt are full
    # c_lo for each token: first column covering d
    for i, d in enumerate(d_sorted):
        c = np.searchsorted(d0, d - 127, side="left")
        c = max(c, nxt)
        while c < C and (fill[c] >= cap or d0[c] + 127 < d):
            if fill[c] >= cap and c == nxt:
                nxt += 1
            c += 1
        if c >= C or d0[c] > d:
            return None
        cols[i] = c
        fill[c] += 1
    return cols


def _prep_edges(cfg, edge_index, edge_weight):
    """Token layout for the matmul-segment-sum propagate.

    Returns (gidx [nc, nwin, ncalls, 128, 64] i16,
             smat [nc, nwin, ncols, 128, 128] bf16,
             sched (stride, ncols, ncalls))."""
    c = cfg
    src = np.asarray(edge_index[0], np.int64)
    dst = np.asarray(edge_index[1], np.int64)
    ew = (np.asarray(edge_weight, np.float32)
          * (1.0 - c.alpha) * (1.0 - c.rsl))          # fold 0.45
    nc, NS, B, nwin = c.ncores, c.NS, c.B, c.nwin
    WN = NS * 2                                       # nodes per window

    per_core = []
    for ci in range(nc):
        m = (dst >= ci * NS) & (dst < (ci + 1) * NS)
        s_c, d_c, w_c = src[m], dst[m] - ci * NS, ew[m]
        wi = s_c // WN
        order = np.lexsort((d_c, wi))
        per_core.append((s_c[order], d_c[order], w_c[order], wi[order]))

    # adaptive shared column schedule (joint earliest-dst sweep per window)
    d0_w, cols_all = [], [[] for _ in range(nc)]
    for w in range(nwin):
        dsts = [per_core[ci][1][per_core[ci][3] == w] for ci in range(nc)]
        ptrs = [0] * nc
        d0l = []
        while True:
            rem = [ci for ci in range(nc) if ptrs[ci] < len(dsts[ci])]
            if not rem:
                break
            a = min(int(dsts[ci][ptrs[ci]]) for ci in rem)
            a = min(a, NS - 128)
            d0l.append(a)
            for ci in rem:
                d = dsts[ci]
                hi = np.searchsorted(d, a + 128, side="left")
                ptrs[ci] = min(ptrs[ci] + 128, hi)
        ncol = ((len(d0l) + 7) // 8) * 8
        d0l += [NS - 128] * (ncol - len(d0l))
        d0a = np.asarray(d0l, np.int64)
        d0_w.append(d0a)
        for ci in range(nc):
            cw = _greedy_cols(dsts[ci], d0a)
            assert cw is not None, f"schedule infeasible w={w} core={ci}"
            cols_all[ci].append(cw)

    ncols_w = [len(d) for d in d0_w]
    ncalls_w = [n // 8 for n in ncols_w]
    col_off = np.concatenate([[0], np.cumsum(ncols_w)])[:nwin + 1]
    call_off = [c // 8 for c in col_off]
    totcol = int(col_off[-1])
    totcall = totcol // 8
    gidx = np.zeros((nc, 128, totcall * 64), np.int16)
    smat = np.zeros((nc, 128, totcol * 128), ml_dtypes.bfloat16)
    for ci in range(nc):
        s_c, d_c, w_c, wi = per_core[ci]
        for w in range(nwin):
            mw = wi == w
            s_w, d_w, w_w = s_c[mw], d_c[mw], w_c[mw]
            cols = cols_all[ci][w]
            d0a = d0_w[w]
            order = np.argsort(cols, kind="stable")
            s_w, d_w, w_w, cols = s_w[order], d_w[order], w_w[order], cols[order]
            slot = np.arange(len(cols)) - np.searchsorted(cols, cols)
            cc = s_w // NS
            rel = (cc - 2 * w) * B + (s_w - cc * NS)
            assert rel.min() >= 0 and rel.max() < 2 * B
            gcol = col_off[w] + cols
            # S: [128 slot-partitions, totcol*128]
            smat[ci].reshape(-1)[
                (slot * totcol + gcol) * 128 + (d_w - d0a[cols])] = w_w
            call = gcol // 8
            t = (gcol % 8) * 128 + slot
            row0, col16 = t % 16, t // 16
            flat = gidx[ci].reshape(-1)
            for g in range(8):
                flat[(row0 + 16 * g) * (totcall * 64) + call * 64 + col16] = \
                    rel.astype(np.int16)
    d0_flat = np.concatenate(d0_w)
    return gidx, smat, (d0_flat, ncalls_w, call_off, col_off, totcall, totcol)
    return gidx, smat, (stride, C, ncalls, d0)


# ----------------------------------------------------------------------
# device program
# ----------------------------------------------------------------------

DBG_LI = 0


def build_nc(cfg):
    c = cfg
    nc = bacc.Bacc(None, target_bir_lowering=False, debug=False,
                   num_swdge_queues=_NQ)
    NT, NS, NTP, B, C_, H, L = c.NT, c.NS, c.NTP, c.B, c.C, c.H, c.L
    D0, NCALLS_W, CALL_OFF, COL_OFF, TOTCALL, TOTCOL = SCHED
    nwin = c.nwin

    def dram_in(name, shape, dt=F32):
        return nc.declare_dram_parameter(name, shape, dt, isOutput=False)

    xin_t = dram_in("xin_t", [H, NS])
    d_t = dram_in("d_t", [NT, 128, C_])
    p_t = dram_in("p_t", [NT, C_, 128], BF16)
    gidx = dram_in("gidx", [128, TOTCALL * 64], I16)
    smat = dram_in("smat", [128, TOTCOL * 128], BF16)
    lin0w = dram_in("lin0w", [H, H])
    lin0b = dram_in("lin0b", [H, 1])
    lin1w = dram_in("lin1w", [H, C_])
    lin1b = dram_in("lin1b", [C_, 1])
    convw = dram_in("convw", [L, H, H])
    cma = dram_in("cma", [C_, C_])
    cmat = dram_in("cmat", [C_, C_])
    i47 = dram_in("i47", [C_, C_])
    ident = dram_in("ident", [128, 128])
    out_t = nc.declare_dram_parameter("out_t", [C_, NS], F32, isOutput=True)

    # internal DRAM
    x_rep = nc.dram_tensor("x_rep", [c.ncores * B, H], BF16,
                           addr_space="Shared")
    x_shp = [nc.dram_tensor(f"x_shp{i}", [B, H], BF16) for i in range(2)]
    x0s_sh = nc.dram_tensor("x0s_sh", [NTP, H], BF16)      # 0.1*x0 node-major
    xinit = [nc.dram_tensor(f"xinit{i}", [H, NTP], F32) for i in range(2)]
    x0sc_fm = nc.dram_tensor("x0sc_fm", [H, NTP], F32)     # 0.1*x0 f-major

    rg = [list(range(c.ncores))]

    def tsize(t):
        return min(128, NS - t * 128)

    with tile.TileContext(nc) as tc:
        nc.gpsimd.load_library(library_config.mlp)
        with (
            tc.tile_pool(name="const", bufs=1) as cpool,
            tc.tile_pool(name="sb", bufs=3) as pool,
            tc.tile_pool(name="gt", bufs=6) as gpool,
            tc.tile_pool(name="st", bufs=6) as spool,
            tc.tile_pool(name="ps", bufs=3, space="PSUM") as psum,
            tc.tile_pool(name="pst", bufs=1, space="PSUM") as pstr,
            tc.tile_pool(name="psc", bufs=3, space="PSUM") as pscol,
            tc.tile_pool(name="psacc", bufs=1, space="PSUM") as psacc,
        ):
            # ---- resident constants ----
            lin0w_sb = cpool.tile([H, H], F32)
            nc.sync.dma_start(lin0w_sb[:], lin0w[:, :])
            lin0b_sb = cpool.tile([H, 1], F32)
            nc.sync.dma_start(lin0b_sb[:], lin0b[:, :])
            lin1w_sb = cpool.tile([H, C_], F32)
            nc.sync.dma_start(lin1w_sb[:], lin1w[:, :])
            lin1b_sb = cpool.tile([C_, 1], F32)
            nc.sync.dma_start(lin1b_sb[:], lin1b[:, :])
            convw_sb = cpool.tile([H, L * H], F32)
            for i in range(L):
                nc.sync.dma_start(convw_sb[:, i * H:(i + 1) * H], convw[i])
            convwb_sb = cpool.tile([H, L * H], BF16)
            nc.vector.tensor_copy(convwb_sb[:], convw_sb[:])
            cma_sb = cpool.tile([C_, C_], F32)
            nc.sync.dma_start(cma_sb[:], cma[:, :])
            cmat_sb = cpool.tile([C_, C_], F32)
            nc.sync.dma_start(cmat_sb[:], cmat[:, :])
            i47_sb = cpool.tile([C_, C_], F32)
            nc.sync.dma_start(i47_sb[:], i47[:, :])
            ident_sb = cpool.tile([128, 128], F32)
            nc.sync.dma_start(ident_sb[:], ident[:, :])
            identb_sb = cpool.tile([128, 128], BF16)
            nc.vector.tensor_copy(identb_sb[:], ident_sb[:])
            lin1wb_sb = cpool.tile([H, C_], BF16)
            nc.vector.tensor_copy(lin1wb_sb[:], lin1w_sb[:])
            b55 = cpool.tile([H, 1], F32)
            nc.vector.tensor_scalar(b55[:], lin0b_sb[:], 0.55, None,
                                    mybir.AluOpType.mult)
            b10 = cpool.tile([H, 1], F32)
            nc.vector.tensor_scalar(b10[:], lin0b_sb[:], 0.1, None,
                                    mybir.AluOpType.mult)
            agg = cpool.tile([H, NTP], F32)
            gi_all = cpool.tile([128, TOTCALL * 64], I16)
            nc.sync.dma_start(gi_all[:], gidx[:, :])

            # ---- lin0: x0 = relu(x @ W0 + b0), double-tile ----
            psA = psacc.tile([C_, H], F32, tag="cen")
            NP = (NT + 1) // 2
            for dt2 in range(NP):
                r0 = dt2 * 256
                W = min(256, NS - r0)
                xi = pool.tile([H, 256], F32, tag="xi")
                nc.sync.dma_start(xi[:, :W], xin_t[:, r0:r0 + W])
                ps0 = psum.tile([H, 256], F32, tag="b")
                nc.tensor.matmul(ps0[:, :W], lin0w_sb[:], xi[:, :W],
                                 start=True, stop=True)
                xi0 = pool.tile([H, 256], F32, tag="xi0")
                nc.scalar.activation(xi0[:, :W], ps0[:, :W],
                                     mybir.ActivationFunctionType.Relu,
                                     bias=b55[:, 0:1], scale=0.55)
                nc.sync.dma_start(xinit[0][:, r0:r0 + W], xi0[:, :W])
                x1 = pool.tile([H, 256], F32, tag="x1")
                nc.scalar.activation(x1[:, :W], ps0[:, :W],
                                     mybir.ActivationFunctionType.Relu,
                                     bias=b10[:, 0:1], scale=0.1)
                nc.sync.dma_start(x0sc_fm[:, r0:r0 + W], x1[:, :W])
                x0b = pool.tile([H, 256], BF16, tag="x0b")
                nc.scalar.activation(x0b[:, :W], ps0[:, :W],
                                     mybir.ActivationFunctionType.Relu,
                                     bias=lin0b_sb[:, 0:1])
                x0n2 = pool.tile([128, 2, H], BF16, tag="x0n")
                nsub = (W + 127) // 128
                for i in range(nsub):
                    P = min(128, W - i * 128)
                    pst = pstr.tile([128, 128], BF16, tag="bb")
                    nc.tensor.transpose(pst[:P, :],
                                        x0b[:, i * 128:i * 128 + P],
                                        identb_sb[:])
                    nc.vector.tensor_copy(x0n2[:P, i, :], pst[:P, :])
                # node-major stores (row = r0 + 128*i + p)
                shp_ap = x_shp[0][r0:r0 + 256, :].rearrange(
                    "(i p) h -> p i h", i=2)
                nc.sync.dma_start(shp_ap, x0n2[:, :, :]) if W == 256 else \
                    nc.sync.dma_start(x_shp[0][r0:r0 + W, :],
                                      x0n2[:, 0, :][:W - 0, :]) \
                    if W <= 128 else [
                        nc.sync.dma_start(x_shp[0][r0:r0 + 128, :],
                                          x0n2[:, 0, :]),
                        nc.sync.dma_start(x_shp[0][r0 + 128:r0 + W, :],
                                          x0n2[:W - 128, 1, :])]
                x0ns = pool.tile([128, 2, H], BF16, tag="x0ns")
                nc.vector.tensor_scalar(x0ns[:, :, :], x0n2[:, :, :], 0.1,
                                        None, mybir.AluOpType.mult)
                x0s_ap = x0s_sh[r0:r0 + 256, :].rearrange(
                    "(i p) h -> p i h", i=2)
                if W == 256:
                    nc.sync.dma_start(x0s_ap, x0ns[:, :, :])
                else:
                    nc.sync.dma_start(x0s_sh[r0:r0 + min(W, 128), :],
                                      x0ns[:min(W, 128), 0, :])
                    if W > 128:
                        nc.sync.dma_start(x0s_sh[r0 + 128:r0 + W, :],
                                          x0ns[:W - 128, 1, :])
                s0 = pool.tile([128, 2, H], F32, tag="s0")
                nc.vector.tensor_scalar(s0[:, :, :], x0n2[:, :, :], 1.1,
                                        None, mybir.AluOpType.mult)
                dtp = pool.tile([128, 2, C_], F32, tag="dt0")
                nc.scalar.dma_start(
                    dtp[:, :nsub, :],
                    d_t[2 * dt2:2 * dt2 + nsub].rearrange(
                        "i p c -> p i c"))
                for i in range(nsub):
                    P = min(128, W - i * 128)
                    t = 2 * dt2 + i
                    nc.tensor.matmul(psA[:], dtp[:P, i, :], s0[:P, i, :],
                                     start=(t == 0), stop=(t == NT - 1))
            cp0 = pool.tile([C_, H], BF16, tag="cp")
            nc.vector.tensor_copy(cp0[:], psA[:])
            nc.sync.dma_start(x_shp[0][NS:NS + C_, :], cp0[:])
            cp0l = pool.tile([C_, H], BF16, tag="cpl")
            nc.vector.tensor_sub(cp0l[:], psA[:], cp0[:])
            nc.sync.dma_start(x_shp[0][NS + C_:NS + 2 * C_, :], cp0l[:])

            # ---- layers ----
            qrr = 0
            for li in range(L):
                cur_shp = x_shp[li % 2]
                nxt_shp = x_shp[(li + 1) % 2]

                nc.gpsimd.collective_compute(
                    "AllGather", mybir.AluOpType.bypass, replica_groups=rg,
                    ins=[cur_shp.ap().opt()], outs=[x_rep.ap().opt()],
                )

                # agg init = 0.45*x + 0.1*x0 (f32, feature-major)
                nc.scalar.dma_start(agg[:, :], xinit[li % 2][:, :])

                # centers from AG payload (hi+lo bf16 pairs -> ~f32)
                cen = pool.tile([C_, H], F32, tag="cen_sb")
                cp_a = pool.tile([C_, H], BF16, tag="cpa")
                nc.sync.dma_start(cp_a[:], x_rep[NS:NS + C_, :])
                cp_b = pool.tile([C_, H], BF16, tag="cpb")
                nc.sync.dma_start(cp_b[:], x_rep[NS + C_:NS + 2 * C_, :])
                nc.vector.tensor_add(cen[:], cp_a[:], cp_b[:])
                for ci in range(1, c.ncores):
                    for half in range(2):
                        cp_i = pool.tile([C_, H], BF16, tag="cpi")
                        r0 = ci * B + NS + half * C_
                        nc.sync.dma_start(cp_i[:], x_rep[r0:r0 + C_, :])
                        nc.vector.tensor_add(cen[:], cen[:], cp_i[:])

                # r_cls from centers (Gram trick); cma pre-scaled by 0.45
                pst = psum.tile([128, 256], F32, tag="b")
                nc.tensor.transpose(pst[:, :C_], cen[:], ident_sb[:C_, :C_])
                cT = pool.tile([128, C_], F32, tag="cT")
                nc.vector.tensor_copy(cT[:], pst[:, :C_])
                psg = psum.tile([128, 256], F32, tag="b")
                nc.tensor.matmul(psg[:C_, :C_], cT[:], cT[:], start=True,
                                 stop=True)
                g = pool.tile([C_, C_], F32, tag="gg")
                nc.vector.tensor_copy(g[:], psg[:C_, :C_])
                gd = pool.tile([C_, C_], F32, tag="gd")
                nc.vector.tensor_mul(gd[:], g[:], i47_sb[:])
                n2 = pool.tile([C_, 1], F32, tag="n2")
                nc.vector.reduce_sum(n2[:], gd[:], AxisListType.X)
                t1 = pool.tile([C_, C_], F32, tag="t1")
                nc.vector.tensor_scalar(t1[:], g[:], -1.0, n2[:, 0:1],
                                        mybir.AluOpType.mult,
                                        mybir.AluOpType.add)
                ps1 = psum.tile([128, 256], F32, tag="b")
                nc.tensor.transpose(ps1[:C_, :C_], t1[:], ident_sb[:C_, :C_])
                nrm = pool.tile([C_, C_], F32, tag="nrm")
                nc.vector.tensor_add(nrm[:], t1[:], ps1[:C_, :C_])
                nc.vector.tensor_relu(nrm[:], nrm[:])
                nc.vector.tensor_add(nrm[:], nrm[:], i47_sb[:])
                rn = pool.tile([C_, C_], F32, tag="rn")
                nc.scalar.sqrt(rn[:], nrm[:])
                inv = pool.tile([C_, C_], F32, tag="inv")
                nc.vector.reciprocal(inv[:], rn[:])
                amat = pool.tile([C_, C_], F32, tag="amat")
                nc.vector.tensor_mul(amat[:], cma_sb[:], inv[:])
                atm = pool.tile([C_, C_], F32, tag="atm")
                nc.vector.tensor_mul(atm[:], cmat_sb[:], inv[:])
                rs = pool.tile([C_, 1], F32, tag="rs")
                nc.vector.reduce_sum(rs[:], amat[:], AxisListType.X)
                psm = psum.tile([128, 256], F32, tag="b")
                nc.tensor.matmul(psm[:C_, :H], atm[:], cen[:], start=True,
                                 stop=True)
                rcls = pool.tile([C_, H], F32, tag="rcls")
                nc.vector.tensor_scalar(rcls[:], cen[:], rs[:, 0:1], None,
                                        mybir.AluOpType.mult)
                nc.vector.tensor_sub(rcls[:], rcls[:], psm[:C_, :H])
                rclsb = pool.tile([C_, H], BF16, tag="rclsb")
                nc.vector.tensor_copy(rclsb[:], rcls[:])

                # - propagate: gather + segment matmul -
                last = li == L - 1

                def emit_passB(t):
                    P = tsize(t)
                    pt = pool.tile([C_, 128], BF16, tag="pt")
                    nc.sync.dma_start(pt[:], p_t[t])
                    ps1b = psum.tile([H, 256], F32, tag="b")
                    nc.tensor.matmul(ps1b[:, :P], rclsb[:], pt[:, :P],
                                     start=True, stop=True)
                    u = pool.tile([H, 128], BF16, tag="u")
                    nc.vector.tensor_add(u[:, :P],
                                         agg[:, t * 128:t * 128 + P],
                                         ps1b[:, :P])
                    ps2 = psum.tile([H, 256], F32, tag="b")
                    nc.tensor.matmul(ps2[:, :P],
                                     convwb_sb[:, li * H:(li + 1) * H],
                                     u[:, :P], start=True, stop=True)
                    xnb = pool.tile([H, 128], BF16, tag="xnb")
                    nc.scalar.activation(xnb[:, :P], ps2[:, :P],
                                         mybir.ActivationFunctionType.Relu)
                    if last:
                        psf = psum.tile([128, 256], F32, tag="b")
                        nc.tensor.matmul(psf[:C_, :P], lin1wb_sb[:],
                                         xnb[:, :P], start=True, stop=True)
                        ot = pool.tile([C_, 128], F32, tag="ot")
                        nc.vector.tensor_scalar(ot[:, :P], psf[:C_, :P],
                                                lin1b_sb[:, 0:1], None,
                                                mybir.AluOpType.add)
                        nc.sync.dma_start(out_t[:, t * 128:t * 128 + P],
                                          ot[:, :P])
                        return
                    xsc = pool.tile([H, 128], F32, tag="xsc")
                    nc.scalar.activation(xsc[:, :P], ps2[:, :P],
                                         mybir.ActivationFunctionType.Relu,
                                         scale=0.45)
                    x0f = pool.tile([H, 128], F32, tag="x0f")
                    nc.scalar.dma_start(x0f[:, :P],
                                        x0sc_fm[:, t * 128:t * 128 + P])
                    xini = pool.tile([H, 128], F32, tag="xini")
                    nc.vector.tensor_add(xini[:, :P], xsc[:, :P], x0f[:, :P])
                    nc.sync.dma_start(xinit[(li + 1) % 2]
                                      [:, t * 128:t * 128 + P], xini[:, :P])
                    psn = pstr.tile([128, 128], BF16, tag="bb")
                    nc.tensor.transpose(psn[:P, :], xnb[:, :P], identb_sb[:])
                    xnn = pool.tile([128, H], BF16, tag="xnn")
                    nc.vector.tensor_copy(xnn[:P, :], psn[:P, :])
                    nc.sync.dma_start(nxt_shp[t * 128:t * 128 + P, :],
                                      xnn[:P, :])
                    x0a = pool.tile([128, H], BF16, tag="x0a")
                    nc.scalar.dma_start(x0a[:P, :],
                                        x0s_sh[t * 128:t * 128 + P, :])
                    sN = pool.tile([128, H], F32, tag="sN")
                    nc.vector.tensor_add(sN[:P, :], xnn[:P, :], x0a[:P, :])
                    dt_ = pool.tile([128, C_], F32, tag="dt")
                    nc.scalar.dma_start(dt_[:P, :], d_t[t][:P, :])
                    nc.tensor.matmul(psA[:], dt_[:P, :], sN[:P, :],
                                     start=(t == 0), stop=(t == NT - 1))

                t_emitted = 0
                for w in range(nwin):
                    wbase = 2 * w * B
                    for k in range(NCALLS_W[w]):
                        cc = CALL_OFF[w] + k
                        if k % 2 == 0:
                            st2 = spool.tile([128, 16, 128], BF16, tag="st")
                            c0 = (COL_OFF[w] + k * 8) * 128
                            nk = min(16, (NCALLS_W[w] - k) * 8)
                            nc.sync.dma_start(
                                st2[:, :nk, :].opt(),
                                smat[:, c0:c0 + nk * 128])
                        st = st2[:, (k % 2) * 8:(k % 2) * 8 + 8, :]
                        gt = gpool.tile([128, 8, 128], BF16, tag="g")
                        nc.gpsimd.dma_gather(
                            gt[:, :, :], x_rep[wbase:wbase + 2 * B, :],
                            gi_all[:, cc * 64:(cc + 1) * 64],
                            num_idxs=1024, num_idxs_reg=1024,
                            elem_size=H, queue_num=qrr % _NQ,
                        )
                        qrr += 1
                        for j in range(8):
                            col = COL_OFF[w] + k * 8 + j
                            d0 = int(D0[col])
                            psC = pscol.tile([H, 128], F32, tag="pc")
                            nc.tensor.matmul(psC[:, :], gt[:, j, :],
                                             st[:, j, :],
                                             start=True, stop=True)
                            nc.vector.tensor_add(agg[:, d0:d0 + 128],
                                                 agg[:, d0:d0 + 128],
                                                 psC[:, :])
                        if w == nwin - 1:
                            nxt_col = (k + 1) * 8
                            if nxt_col >= NCALLS_W[w] * 8:
                                t_done = NT - 1
                            else:
                                t_done = int(D0[COL_OFF[w] + nxt_col]) \
                                    // 128 - 1
                            while t_emitted <= min(t_done, NT - 1):
                                emit_passB(t_emitted)
                                t_emitted += 1
                if not last:
                    cpn = pool.tile([C_, H], BF16, tag="cp")
                    nc.vector.tensor_copy(cpn[:], psA[:])
                    nc.sync.dma_start(nxt_shp[NS:NS + C_, :], cpn[:])
                    cpnl = pool.tile([C_, H], BF16, tag="cpl")
                    nc.vector.tensor_sub(cpnl[:], psA[:], cpn[:])
                    nc.sync.dma_start(nxt_shp[NS + C_:NS + 2 * C_, :],
                                      cpnl[:])

    nc.compile()
    return nc


# ----------------------------------------------------------------------
# host wrapper
# ----------------------------------------------------------------------

def _prep_inputs(cfg, inputs):
    c = cfg
    x = np.asarray(inputs["x"], np.float32)
    label = np.asarray(inputs["label"], np.int64)
    p = np.asarray(inputs["p"], np.float32)
    cm = np.asarray(inputs["cm"], np.float32)
    lin0_w = np.asarray(inputs["lin0_w"], np.float32)
    lin0_b = np.asarray(inputs["lin0_b"], np.float32)
    lin1_w = np.asarray(inputs["lin1_w"], np.float32)
    lin1_b = np.asarray(inputs["lin1_b"], np.float32)
    conv_w = np.asarray(inputs["conv_w"], np.float32)

    gidx, smat, sched = _prep_edges(cfg, inputs["edge_index"],
                                    inputs["edge_weight"])

    cnt = np.bincount(label, minlength=c.C).astype(np.float32)
    cnt = np.maximum(cnt, 1.0)
    cma = cm[:, 0, :] * (1.0 - c.alpha) * c.rsl      # fold 0.45 into r path
    i47 = np.eye(c.C, dtype=np.float32)
    ident = np.eye(128, dtype=np.float32)
    # fold beta mix into conv weights
    convw2 = np.stack([
        (1.0 - b) * np.eye(c.H, dtype=np.float32) + b * conv_w[i]
        for i, b in enumerate(
            float(np.log(c.theta / (i + 1) + 1.0)) for i in range(c.L))
    ])

    in_maps = []
    for ci in range(c.ncores):
        r0 = ci * c.NS
        xs = x[r0:r0 + c.NS]
        lab = label[r0:r0 + c.NS]
        ps = p[r0:r0 + c.NS]
        d_tile = np.zeros((c.NTP, c.C), np.float32)
        d_tile[np.arange(c.NS), lab] = 1.0 / cnt[lab]
        p_pad = np.zeros((c.NTP, c.C), np.float32)
        p_pad[:c.NS] = ps
        in_maps.append({
            "xin_t": np.ascontiguousarray(xs.T),
            "d_t": np.ascontiguousarray(d_tile.reshape(c.NT, 128, c.C)),
            "p_t": np.ascontiguousarray(
                p_pad.reshape(c.NT, 128, c.C).transpose(0, 2, 1)).astype(
                    ml_dtypes.bfloat16),
            "gidx": gidx[ci], "smat": smat[ci],
            "lin0w": lin0_w, "lin0b": lin0_b.reshape(-1, 1),
            "lin1w": lin1_w, "lin1b": lin1_b.reshape(-1, 1),
            "convw": convw2, "cma": cma,
            "cmat": np.ascontiguousarray(cma.T),
            "i47": i47, "ident": ident,
        })
    return in_maps, sched


_BUILT = {}


def kernel(**inputs):
    cfg = DEF
    global SCHED
    in_maps, sched = _prep_inputs(cfg, inputs)
    key = "default"
    if key not in _BUILT:
        SCHED = sched
        _BUILT[key] = build_nc(cfg)
    nc = _BUILT[key]
    res = bass_utils.run_bass_kernel_spmd(nc, in_maps,
                                          core_ids=list(range(cfg.ncores)))
    outs = [res.results[ci]["out_t"].T for ci in range(cfg.ncores)]
    return np.ascontiguousarray(np.concatenate(outs, 0))
